# revision 2
# baseline (speedup 1.0000x reference)
"""Trainium2 Bass kernel for nn_LookAtMappingNetwork (gnn_message_passing).

Strategy
--------
The module's output only reads the final node features at rows R = {i*250 :
i in 0..63} (``ws = x[::250]``).  Working backwards through the two message
-passing processors, only a small data-dependent subset of edges/nodes can
influence those rows, for ANY edge_index:

    E1 = edges with dst in R          (~6 per graph)   -> proc-1 edge MLP
    S  = R  ∪  src[E1]                (~65 per core)   -> rows where x1 needed
    E0 = edges with dst in S          (~375 per core)  -> proc-0 edge MLP

Segment-mean counts stay exact because E0/E1 contain ALL edges landing on
S/R.  Everything else the reference computes is dead code.  Each of the 8
cores handles 8 output rows (its R_c) fully independently; weights are
replicated and streamed from HBM through a small rotating SBUF pool.  All
floating-point math runs on device; the host only does integer index-set
construction (sharding/marshalling).

Layout: proc-0 edge layer 1 runs feature-major (z-contributions arrive via
64-wide selection matmuls, look-at contributions via one packed K=65
matmul); all later layers run token-major (tokens<=128 on partitions, 512
output features on the free axis) with the layer bias injected as an extra
K=1 matmul row, so every heavy matmul streams a 512-wide moving operand.
Matmul operands are bf16 (weights cast host-side, activations cast by the
ACT engine on write; fp32 PSUM accumulation), giving single-pass PE
matmuls and halved weight-stream DMA.  leaky_relu(0.2) is composed from
Identity+Relu activations (the HW Lrelu LUT bakes alpha=0.01).  Weight
K-tile pairs/quads share one DMA, alternated across both HWDGE rings.
"""

import math

import ml_dtypes
import numpy as np

import concourse.bacc as bacc
import concourse.bass as bass
import concourse.mybir as mybir
import concourse.tile as tile
from concourse.bass import IndirectOffsetOnAxis
from concourse.bass_utils import run_bass_kernel_spmd
from concourse.masks import make_identity

f32 = mybir.dt.float32
fr = mybir.dt.bfloat16
i32 = mybir.dt.int32
AF = mybir.ActivationFunctionType
OP = mybir.AluOpType

NV = 250
B = 64
D = 512
LR = 0.01
SQ2 = math.sqrt(2.0)
N_CORES = 8
R_PER = B // N_CORES  # output rows per core

CAP_E0 = 384
CAP_S = 128
CAP_E1 = 128

G_E00 = LR / math.sqrt(1034.0)
G_E01 = LR / math.sqrt(512.0)
G_N00 = LR / math.sqrt(1030.0)
G_N01 = LR / math.sqrt(512.0)
G_E10 = LR / math.sqrt(1536.0)
G_E11 = LR / math.sqrt(512.0)
G_N10 = LR / math.sqrt(1024.0)
G_N11 = LR / math.sqrt(512.0)

# agg0 feature splits for the proc-0 node MLP:  [la_mean(3) | ef0_mean(512)]
AGG0_SPLITS = [(0, 3), (3, 131), (131, 259), (259, 387), (387, 515)]


def _build_program():
    """Emit the per-core Bass program (SPMD across 8 cores)."""
    nc = bacc.Bacc("TRN2", target_bir_lowering=False, debug=False,
                   enable_asserts=False, num_devices=N_CORES)

    def din(name, shape, dtype=fr):
        return nc.dram_tensor(name, shape, dtype, kind="ExternalInput")

    z_d = din("z", [B, D], f32)
    la_d = din("lookats", [16000, 3], f32)
    w0e0_zsrc = din("w0e0_zsrc", [512, 512])
    w0e0_zdst = din("w0e0_zdst", [512, 512])
    w0e0_laA = din("w0e0_laA", [3, 512])
    w0e0_laB = din("w0e0_laB", [3, 512])
    w0e0_rel = din("w0e0_rel", [3, 512])
    w0e0_wd = din("w0e0_wd", [1, 512])
    w0e1 = din("w0e1", [512, 512])
    w0n0_z = din("w0n0_z", [512, 512])
    w0n0_la = din("w0n0_la", [3, 512])
    w0n0_agg = din("w0n0_agg", [515, 512])
    w0n1 = din("w0n1", [512, 512])
    w1e0 = din("w1e0", [1536, 512])
    w1e1 = din("w1e1", [512, 512])
    w1n0 = din("w1n0", [1024, 512])
    w1n1 = din("w1n1", [512, 512])
    biases = {k: din("b_" + k, [512], f32) for k in
              ["e00", "e01", "n00", "n01", "e10", "e11", "n10", "n11"]}
    e0_src_d = din("e0_src", [CAP_E0, 1], i32)
    e0_dst_d = din("e0_dst", [CAP_E0, 1], i32)
    e0_srcmod_d = din("e0_srcmod", [64, CAP_E0], f32)
    e0_dstmod_d = din("e0_dstmod", [64, CAP_E0], f32)
    e0_sigma_d = din("e0_sigma", [CAP_E0], f32)
    s_node_d = din("s_node", [CAP_S, 1], i32)
    s_mod_d = din("s_mod", [64, CAP_S], f32)
    e1_pos_d = din("e1_pos", [128, CAP_E1], f32)
    e1_srcslot_d = din("e1_srcslot", [128, CAP_E1], f32)
    e1_dstslot_d = din("e1_dstslot", [128, CAP_E1], f32)
    e1_sigma_d = din("e1_sigma", [CAP_E1], f32)

    out_d = nc.dram_tensor("out", [R_PER, 14, D], f32, kind="ExternalOutput")

    NT0 = CAP_E0 // 128  # e-tiles in proc-0 edge set
    k4 = [(0, 128), (128, 256), (256, 384), (384, 512)]

    with tile.TileContext(nc) as tc, \
            tc.tile_pool(name="w", bufs=1) as wp, \
            tc.tile_pool(name="wk", bufs=8) as wk, \
            tc.tile_pool(name="tmp", bufs=10) as tp, \
            tc.tile_pool(name="psb", bufs=3, space="PSUM") as psb, \
            tc.tile_pool(name="pss", bufs=5, space="PSUM") as pss:

        # ---------------- constants ----------------
        ident_f = wp.tile([128, 128], f32, name="ident_f")
        make_identity(nc, ident_f[:])
        ident = wp.tile([128, 128], fr, name="ident")
        nc.vector.tensor_copy(ident[:], ident_f[:])
        idents = {fr: ident, f32: ident_f}
        ones_f32 = wp.tile([128, 1], f32, name="ones_f32")
        nc.gpsimd.memset(ones_f32[:], 1.0)
        iota_free = wp.tile([128, 128], f32, name="iota_free")
        nc.gpsimd.iota(iota_free[:], pattern=[[1, 128]], base=0,
                       channel_multiplier=0, allow_small_or_imprecise_dtypes=True)
        iota_part = []
        for t in range(NT0):
            it = wp.tile([128, 1], f32, name=f"iota_part{t}")
            nc.gpsimd.iota(it[:], pattern=[[1, 1]], base=128 * t,
                           channel_multiplier=1,
                           allow_small_or_imprecise_dtypes=True)
            iota_part.append(it)
        ones_row = wp.tile([1, 128], fr, name="ones_row")
        nc.vector.tensor_copy(ones_row[:], ones_f32[:1, :1].to_broadcast([1, 128]))


        _uid = [0]

        def uid():
            _uid[0] += 1
            return _uid[0]

        def sb(shape, name):
            return wp.tile(shape, fr, name=name)

        _dma_rr = [0]

        def wdma(out_ap, in_ap):
            # Alternate the two HWDGE rings (SP and ACT) so weight streaming
            # isn't serialized on one ring.
            eng = nc.sync if _dma_rr[0] % 2 == 0 else nc.scalar
            _dma_rr[0] += 1
            eng.dma_start(out_ap, in_ap)

        def wtile(dram_t, a, b_):
            t = wk.tile([b_ - a, 512], fr, name=f"wt{uid()}", tag="wk")
            wdma(t[:], dram_t[a:b_, :])
            return t

        def wtilen(dram_t, a, n):
            """Load rows [a, a+128n) as one DMA -> n K-tile views."""
            t = wk.tile([128, n, 512], fr, name=f"wt{uid()}", tag=f"wk{n}")
            wdma(t[:], dram_t[a:a + 128 * n, :].rearrange("(t p) d -> p t d",
                                                          p=128))
            return [t[:, j, :] for j in range(n)]

        def wtiles_for(dram_t, rows):
            """K-tiles for row ranges; runs of adjacent 128-rows share a DMA."""
            tiles = []
            i = 0
            while i < len(rows):
                a, b_ = rows[i]
                run = 0
                while (run < 4 and i + run < len(rows)
                       and rows[i + run] == (a + 128 * run, a + 128 * (run + 1))):
                    run += 1
                if run >= 2:
                    tiles.extend(wtilen(dram_t, a, run))
                    i += run
                else:
                    tiles.append(wtile(dram_t, a, b_)[:])
                    i += 1
            return tiles

        def copyT(src_ap, p, f, dst_ap):
            """PE transpose src [p, f] -> existing sbuf dst_ap [f, p]."""
            sdt = src_ap.dtype
            ps = pss.tile([f, p], sdt, name=f"psT{uid()}", tag="pssm")
            nc.tensor.transpose(ps[:], src_ap, idents[sdt][:p, :p])
            nc.vector.tensor_copy(dst_ap, ps[:])

        def peT(src_ap, p, f, name):
            dst = sb([f, p], name)
            copyT(src_ap, p, f, dst[:])
            return dst

        def brow(key, gain):
            """Bias as a K=1 matmul row: (LR/gain) * b, shape [1, 512]."""
            raw = tp.tile([1, 512], f32, name=f"braw{uid()}", tag="yaf")
            nc.sync.dma_start(raw[:], biases[key][None, :])
            t = wp.tile([1, 512], fr, name=f"brow_{key}")
            nc.vector.tensor_scalar_mul(t[:], raw[:], LR / gain)
            return t

        def lrelu_tok(psum_ap, gain, out_ap, p, n):
            """out = sqrt2*leaky_relu(gain*acc, 0.2); bias already in acc."""
            odt = out_ap.dtype
            ya = tp.tile([p, n], odt, name=f"ya{uid()}",
                         tag="ya" if odt == fr else "yaf")
            nc.scalar.activation(ya[:], psum_ap, AF.Identity,
                                 bias=0.0, scale=0.2 * SQ2 * gain)
            nc.scalar.activation(out_ap, psum_ap, AF.Relu,
                                 bias=0.0, scale=0.8 * SQ2 * gain)
            nc.vector.tensor_add(out_ap, out_ap, ya[:])

        def tok_layer(lhsT_aps, wspec, brow_t, gain, out_ap, p):
            """Token-major FC layer: out[p tokens, 512] = lrelu(in @ W^T + b).

            lhsT_aps: feature-major input K-tiles [k_i, p tokens].
            wspec: matching (dram, row_a, row_b) K-tiles of W^T [K, 512].
            """
            ps = psb.tile([p, 512], f32, name=f"psL{uid()}", tag="psbig")
            wts = wtiles_for(wspec[0][0], [(a, b_) for _, a, b_ in wspec]) \
                if all(w[0] is wspec[0][0] for w in wspec) else None
            for k, ((dt_, a, b_), lh) in enumerate(zip(wspec, lhsT_aps)):
                wt = wts[k] if wts is not None else wtile(dt_, a, b_)[:]
                nc.tensor.matmul(ps[:], lh, wt, start=(k == 0), stop=False)
            nc.tensor.matmul(ps[:], ones_row[:, :p], brow_t[:],
                             start=False, stop=True)
            lrelu_tok(ps[:], gain, out_ap, p, 512)
            return ps

        # ---------------- z normalization ----------------
        zt = tp.tile([64, 512], f32, name="zt", tag="yaf")
        nc.sync.dma_start(zt[:], z_d[:, :])
        zsq = tp.tile([64, 512], f32, name="zsq", tag="rrf")
        nc.vector.tensor_tensor(zsq[:], zt[:], zt[:], op=OP.mult)
        zss = wp.tile([64, 1], f32, name="zss")
        nc.vector.tensor_reduce(zss[:], zsq[:], axis=mybir.AxisListType.X, op=OP.add)
        nc.vector.tensor_scalar(zss[:], zss[:], 1.0 / 512.0, 1e-8, OP.mult, OP.add)
        zsr = wp.tile([64, 1], f32, name="zsr")
        nc.scalar.sqrt(zsr[:], zss[:])
        zrin = wp.tile([64, 1], f32, name="zrin")
        nc.vector.reciprocal(zrin[:], zsr[:])
        znt = sb([64, 512], "znt")          # zn, token-major [64 z, 512 f]
        nc.vector.tensor_scalar_mul(znt[:], zt[:], zrin[:, :1])

        znT = []                            # zn^T feature-major, 4x [128, 64]
        for k in range(4):
            znT.append(peT(znt[:64, 128 * k:128 * (k + 1)], 64, 128, f"znT{k}"))

        # ---------------- proc-0 edge gathers ----------------
        la_src, la_dst, dist, sigma = [], [], [], []
        for t in range(NT0):
            ixs = wp.tile([128, 1], i32, name=f"ixs{t}")
            nc.sync.dma_start(ixs[:], e0_src_d[128 * t:128 * (t + 1), :])
            ixd = wp.tile([128, 1], i32, name=f"ixd{t}")
            nc.sync.dma_start(ixd[:], e0_dst_d[128 * t:128 * (t + 1), :])
            ls = wp.tile([128, 3], f32, name=f"lasrc{t}")
            nc.gpsimd.indirect_dma_start(
                out=ls[:], out_offset=None, in_=la_d[:],
                in_offset=IndirectOffsetOnAxis(ap=ixs[:, :1], axis=0))
            ld = wp.tile([128, 3], f32, name=f"ladst{t}")
            nc.gpsimd.indirect_dma_start(
                out=ld[:], out_offset=None, in_=la_d[:],
                in_offset=IndirectOffsetOnAxis(ap=ixd[:, :1], axis=0))
            la_src.append(ls)
            la_dst.append(ld)
            dd = tp.tile([128, 3], f32, name=f"dd{t}", tag="yaf")
            nc.vector.tensor_tensor(dd[:], ld[:], ls[:], op=OP.subtract)
            nc.vector.tensor_tensor(dd[:], dd[:], dd[:], op=OP.mult)
            ds = tp.tile([128, 1], f32, name=f"ds{t}", tag="rr")
            nc.vector.tensor_reduce(ds[:], dd[:], axis=mybir.AxisListType.X,
                                    op=OP.add)
            dt_ = wp.tile([128, 1], f32, name=f"dist{t}")
            nc.scalar.sqrt(dt_[:], ds[:])
            dist.append(dt_)
            sg = wp.tile([128, 1], f32, name=f"sigma{t}")
            nc.sync.dma_start(sg[:], e0_sigma_d[128 * t:128 * (t + 1), None])
            sigma.append(sg)

        smod_f = tp.tile([64, CAP_S], f32, name="smod_f", tag="yaf")
        nc.sync.dma_start(smod_f[:], s_mod_d[:, :])
        selS = sb([64, CAP_S], "selS")
        nc.vector.tensor_scalar(selS[:], smod_f[:], iota_part[0][:64, :1], None,
                                OP.is_equal)
        zgS = []
        for c in range(4):
            ps = pss.tile([128, CAP_S], f32, name=f"ps_zg{c}", tag="pssm")
            nc.tensor.matmul(ps[:], znt[:64, 128 * c:128 * (c + 1)], selS[:],
                             start=True, stop=True)
            t_ = sb([128, CAP_S], f"zgS{c}")
            nc.vector.tensor_copy(t_[:], ps[:])
            zgS.append(t_)
        s_ix = wp.tile([CAP_S, 1], i32, name="s_ix")
        nc.sync.dma_start(s_ix[:], s_node_d[:, :])
        laS = wp.tile([CAP_S, 3], f32, name="laS")
        nc.gpsimd.indirect_dma_start(
            out=laS[:], out_offset=None, in_=la_d[:],
            in_offset=IndirectOffsetOnAxis(ap=s_ix[:, :1], axis=0))
        laST = peT(laS[:], CAP_S, 3, "laST")

        # zterm_A/B [64 z, 512 dout], token-major (no activation, no bias)
        def zterm(dram_t, name):
            ps = psb.tile([64, 512], f32, name=f"ps_{name}", tag="psbig")
            wts = wtiles_for(dram_t, k4)
            for k in range(4):
                nc.tensor.matmul(ps[:], znT[k][:], wts[k],
                                 start=(k == 0), stop=(k == 3))
            t = sb([64, 512], name)
            nc.vector.tensor_copy(t[:], ps[:])
            return t

        ztermA = zterm(w0e0_zsrc, "ztermA")
        ztermB = zterm(w0e0_zdst, "ztermB")

        # edge-encoder look-at weight combos (rel folds into src/dst parts)
        laA = wtile(w0e0_laA, 0, 3)
        laB = wtile(w0e0_laB, 0, 3)
        rel = wtile(w0e0_rel, 0, 3)
        wd = wtile(w0e0_wd, 0, 1)
        # Pack the three look-at weight blocks into one K=65 lhsT tile at
        # 32-aligned partition offsets (0: laA-rel, 32: laB+rel, 64: wd);
        # gap rows are zero-filled so they contribute nothing.
        zeros_f32 = wp.tile([128, 1], f32, name="zeros_f32")
        nc.gpsimd.memset(zeros_f32[:], 0.0)
        laWc = sb([65, 512], "laWc")
        nc.vector.tensor_copy(laWc[:], zeros_f32[:65, :1].to_broadcast([65, 512]))
        nc.vector.tensor_tensor(laWc[0:3, :], laA[:], rel[:], op=OP.subtract)
        nc.vector.tensor_tensor(laWc[32:35, :], laB[:], rel[:], op=OP.add)
        nc.vector.tensor_copy(laWc[64:65, :], wd[:])

        # feature-major rhs for the la terms, matching laWc's row layout:
        # assemble [128, 65] (cols 0:3 la_src, 32:35 la_dst, 64 dist) and do
        # ONE transpose per e-tile instead of three.
        laRhs = sb([65, CAP_E0], "laRhs")
        for t in range(NT0):
            cmb = tp.tile([128, 65], f32, name=f"lacmb{t}", tag="yaf")
            nc.vector.tensor_copy(cmb[:],
                                  zeros_f32[:, :1].to_broadcast([128, 65]))
            nc.vector.tensor_copy(cmb[:, 0:3], la_src[t][:])
            nc.vector.tensor_copy(cmb[:, 32:35], la_dst[t][:])
            nc.vector.tensor_copy(cmb[:, 64:65], dist[t][:])
            copyT(cmb[:], 128, 65, laRhs[:, 128 * t:128 * (t + 1)])

        # z-index selection matrices [64, E0]
        srcmod_f = tp.tile([64, CAP_E0], f32, name="srcmod_f", tag="yaf")
        nc.sync.dma_start(srcmod_f[:], e0_srcmod_d[:, :])
        dstmod_f = tp.tile([64, CAP_E0], f32, name="dstmod_f", tag="rrf")
        nc.sync.dma_start(dstmod_f[:], e0_dstmod_d[:, :])
        sel0s = sb([64, CAP_E0], "sel0s")
        sel0d = sb([64, CAP_E0], "sel0d")
        nc.vector.tensor_scalar(sel0s[:], srcmod_f[:], iota_part[0][:64, :1], None,
                                OP.is_equal)
        nc.vector.tensor_scalar(sel0d[:], dstmod_f[:], iota_part[0][:64, :1], None,
                                OP.is_equal)

        # ------------- proc-0 edge MLP layer 1 (feature-major) ------------
        # h0 chunks [128 dout, E0]; bias via per-partition AP on the ACT.
        b_e00_1 = wp.tile([128, 4], f32, name="b_e00_1")
        b_e00_2 = wp.tile([128, 4], f32, name="b_e00_2")
        braw00 = tp.tile([128, 4], f32, name="braw00", tag="yaf")
        nc.sync.dma_start(braw00[:], biases["e00"][:].rearrange("(c p) -> p c", p=128))
        nc.vector.tensor_scalar_mul(b_e00_1[:], braw00[:], 0.2 * SQ2 * LR)
        nc.vector.tensor_scalar_mul(b_e00_2[:], braw00[:], 0.8 * SQ2 * LR)

        h0 = []
        for c in range(4):
            cs = slice(128 * c, 128 * (c + 1))
            ps = psb.tile([128, CAP_E0], f32, name=f"ps_efp{c}", tag="psbig")
            nc.tensor.matmul(ps[:], ztermA[:64, cs], sel0s[:], start=True, stop=False)
            nc.tensor.matmul(ps[:], ztermB[:64, cs], sel0d[:], start=False, stop=False)
            nc.tensor.matmul(ps[:], laWc[:, cs], laRhs[:], start=False, stop=True)
            o = sb([128, CAP_E0], f"h0_{c}")
            ya = tp.tile([128, CAP_E0], fr, name=f"ya0{c}", tag="ya")
            nc.scalar.activation(ya[:], ps[:], AF.Identity,
                                 bias=b_e00_1[:, c:c + 1], scale=0.2 * SQ2 * G_E00)
            nc.scalar.activation(o[:], ps[:], AF.Relu,
                                 bias=b_e00_2[:, c:c + 1], scale=0.8 * SQ2 * G_E00)
            nc.vector.tensor_add(o[:], o[:], ya[:])
            h0.append(o)

        # ------------- proc-0 edge MLP layer 2 (token-major) --------------
        # ef0 written straight into msg tiles: [la_dst(3) | ef0(512) | 1]
        brow_e01 = brow("e01", G_E01)
        w0e1t = wtiles_for(w0e1, k4)
        msg = []
        for t in range(NT0):
            m = sb([128, 516], f"msg{t}")
            nc.vector.tensor_copy(m[:, 0:3], la_dst[t][:])
            nc.vector.tensor_copy(m[:, 515:516], ones_f32[:, :1])
            es = slice(128 * t, 128 * (t + 1))
            ps = psb.tile([128, 512], f32, name=f"ps_ef0{t}", tag="psbig")
            for k in range(4):
                nc.tensor.matmul(ps[:], h0[k][:, es], w0e1t[k],
                                 start=(k == 0), stop=False)
            nc.tensor.matmul(ps[:], ones_row[:, :128], brow_e01[:],
                             start=False, stop=True)
            lrelu_tok(ps[:], G_E01, m[:, 3:515], 128, 512)
            msg.append(m)

        # ---------------- aggregation onto S ----------------
        G0 = []
        for t in range(NT0):
            g = sb([128, 128], f"G0_{t}")
            nc.vector.tensor_scalar(g[:], iota_free[:], sigma[t][:, :1], None,
                                    OP.is_equal)
            G0.append(g)

        ps_a = psb.tile([128, 512], f32, name="ps_agg0a", tag="psbig")
        ps_b = pss.tile([128, 4], f32, name="ps_agg0b", tag="pssm")
        for t in range(NT0):
            nc.tensor.matmul(ps_a[:], G0[t][:], msg[t][:, 0:512],
                             start=(t == 0), stop=(t == NT0 - 1))
            nc.tensor.matmul(ps_b[:], G0[t][:], msg[t][:, 512:516],
                             start=(t == 0), stop=(t == NT0 - 1))
        cnt = wp.tile([128, 1], f32, name="cnt")
        nc.vector.tensor_scalar(cnt[:], ps_b[:, 3:4], 1.0, None, OP.max)
        rin = wp.tile([128, 1], f32, name="rin")
        nc.vector.reciprocal(rin[:], cnt[:])
        # msg feature order is [la(3) | ef(512)], so cols 0:512 of ps_a plus
        # cols 0:3 of ps_b form the contiguous 515-wide [la_mean | ef_mean].
        aggtok = sb([128, 515], "aggtok")   # [S slot, (la_mean|ef_mean)]
        nc.vector.tensor_scalar_mul(aggtok[:, 0:512], ps_a[:, 0:512], rin[:, :1])
        nc.vector.tensor_scalar_mul(aggtok[:, 512:515], ps_b[:, 0:3], rin[:, :1])
        aggT = []
        for j, (a, b_) in enumerate(AGG0_SPLITS):
            aggT.append(peT(aggtok[:, a:b_], 128, b_ - a, f"aggT{j}"))

        # ---------------- node MLP 0 -> x1 (token-major, S slots) ---------
        hn_tok = sb([CAP_S, 512], "hn_tok")
        tok_layer(
            [zgS[k][:] for k in range(4)] + [laST[:]] +
            [aggT[j][:] for j in range(5)],
            [(w0n0_z, a, b_) for a, b_ in k4] + [(w0n0_la, 0, 3)] +
            [(w0n0_agg, a, b_) for a, b_ in AGG0_SPLITS],
            brow("n00", G_N00), G_N00, hn_tok[:], CAP_S)

        hnT = []
        for c in range(4):
            hnT.append(peT(hn_tok[:, 128 * c:128 * (c + 1)], CAP_S, 128,
                           f"hnT{c}"))
        x1tok = sb([CAP_S, 512], "x1tok")
        tok_layer([hnT[k][:] for k in range(4)],
                  [(w0n1, a, b_) for a, b_ in k4],
                  brow("n01", G_N01), G_N01, x1tok[:], CAP_S)

        # x1 at the R slots, feature-major [128 f, 8], via identity columns
        x1R = []
        for c in range(4):
            ps = pss.tile([128, R_PER], f32, name=f"ps_x1R{c}", tag="pssm")
            nc.tensor.matmul(ps[:], x1tok[:, 128 * c:128 * (c + 1)],
                             ident[:CAP_S, 0:R_PER], start=True, stop=True)
            t_ = sb([128, R_PER], f"x1R{c}")
            nc.vector.tensor_copy(t_[:], ps[:])
            x1R.append(t_)

        # ---------------- proc-1 edge MLP (token-major, E1) ---------------
        def load_sel(dram_t, name, nt=1):
            raw = tp.tile([128, CAP_E1], f32, name=f"{name}raw", tag="yaf")
            nc.sync.dma_start(raw[:], dram_t[:, :])
            sels = []
            for t in range(nt):
                s_ = sb([128, CAP_E1], f"{name}{t}")
                nc.vector.tensor_scalar(s_[:], raw[:], iota_part[t][:, :1],
                                        None, OP.is_equal)
                sels.append(s_)
            return sels

        selA = load_sel(e1_srcslot_d, "selA")[0]
        selB = load_sel(e1_dstslot_d, "selB")[0]
        selE = load_sel(e1_pos_d, "selE", nt=NT0)

        def sel_gather(lhsT_fns, sel_tiles, name, n=CAP_E1):
            outs = []
            for c in range(4):
                ps = pss.tile([128, n], f32, name=f"ps_{name}{c}", tag="pssm")
                for t, s_ in enumerate(sel_tiles):
                    nc.tensor.matmul(ps[:], lhsT_fns[t](c), s_[:],
                                     start=(t == 0), stop=(t == len(sel_tiles) - 1))
                o = sb([128, n], f"{name}{c}")
                nc.vector.tensor_copy(o[:], ps[:])
                outs.append(o)
            return outs

        x1gA = sel_gather([lambda c: x1tok[:, 128 * c:128 * (c + 1)]], [selA], "x1gA")
        x1gB = sel_gather([lambda c: x1tok[:, 128 * c:128 * (c + 1)]], [selB], "x1gB")
        ef0g = sel_gather(
            [(lambda t: (lambda c: msg[t][:, 3 + 128 * c:3 + 128 * (c + 1)]))(t)
             for t in range(NT0)], selE, "ef0g")

        h1tok = sb([CAP_E1, 512], "h1tok")
        tok_layer([r[:] for r in (x1gA + x1gB + ef0g)],
                  [(w1e0, 128 * i, 128 * (i + 1)) for i in range(12)],
                  brow("e10", G_E10), G_E10, h1tok[:], CAP_E1)

        h1T = []
        for c in range(4):
            h1T.append(peT(h1tok[:, 128 * c:128 * (c + 1)], CAP_E1, 128,
                           f"h1T{c}"))
        # ef1 written straight into msg1 cols 0:512 (token-major already)
        msg1 = sb([CAP_E1, 514], "msg1")
        nc.vector.tensor_copy(msg1[:, 512:514],
                              ones_f32[:, 0:1].to_broadcast([128, 2]))
        tok_layer([h1T[k][:] for k in range(4)],
                  [(w1e1, a, b_) for a, b_ in k4],
                  brow("e11", G_E11), G_E11, msg1[:, 0:512], CAP_E1)

        # ---------------- aggregation onto R (8 rows) ---------------------
        e1sig = wp.tile([CAP_E1, 1], f32, name="e1sig")
        nc.sync.dma_start(e1sig[:], e1_sigma_d[:, None])
        G1 = sb([CAP_E1, R_PER], "G1")
        nc.vector.tensor_scalar(G1[:], iota_free[:, 0:R_PER], e1sig[:, :1], None,
                                OP.is_equal)
        ps1 = psb.tile([R_PER, 512], f32, name="ps_agg1", tag="psbig")
        nc.tensor.matmul(ps1[:], G1[:], msg1[:, 0:512], start=True, stop=True)
        ps2 = pss.tile([R_PER, 2], f32, name="ps_agg1b", tag="pssm")
        nc.tensor.matmul(ps2[:], G1[:], msg1[:, 512:514], start=True, stop=True)
        cnt1 = wp.tile([R_PER, 1], f32, name="cnt1")
        nc.vector.tensor_scalar(cnt1[:], ps2[:, 0:1], 1.0, None, OP.max)
        rin1 = wp.tile([R_PER, 1], f32, name="rin1")
        nc.vector.reciprocal(rin1[:], cnt1[:])
        agg1tok = sb([R_PER, 512], "agg1tok")
        nc.vector.tensor_scalar_mul(agg1tok[:], ps1[:], rin1[:, :1])
        agg1T = []
        for c in range(4):
            agg1T.append(peT(agg1tok[:R_PER, 128 * c:128 * (c + 1)], R_PER, 128,
                             f"agg1T{c}"))

        # ---------------- final node MLP (token-major, 8 rows) ------------
        hftok = sb([R_PER, 512], "hftok")
        tok_layer([x1R[k][:] for k in range(4)] + [agg1T[k][:] for k in range(4)],
                  [(w1n0, 128 * i, 128 * (i + 1)) for i in range(8)],
                  brow("n10", G_N10), G_N10, hftok[:], R_PER)
        hfT = []
        for c in range(4):
            hfT.append(peT(hftok[:R_PER, 128 * c:128 * (c + 1)], R_PER, 128,
                           f"hfT{c}"))
        wstok = wp.tile([R_PER, 512], f32, name="wstok")
        tok_layer([hfT[k][:] for k in range(4)],
                  [(w1n1, a, b_) for a, b_ in k4],
                  brow("n11", G_N11), G_N11, wstok[:], R_PER)

        nc.sync.dma_start(out_d[:, :, :],
                          wstok[:, None, :].to_broadcast([R_PER, 14, 512]))


        # PE "heater": a dependency-free chain of tiny bf16 matmuls, emitted
        # last so the Tile scheduler drops them into PE idle gaps.  Keeping
        # the PE array active holds the HAM clock gate at K=8/8 (2.4 GHz);
        # without this the inter-layer dependency stalls re-throttle the PE
        # to 1.2 GHz and every real matmul runs at half rate.
        N_HEAT = 160
        if N_HEAT:
            hseed = wp.tile([32, 256], fr, name="hseed")
            nc.vector.tensor_copy(hseed[:, 0:128], ident[:32, :128])
            nc.vector.tensor_copy(hseed[:, 128:256], ident[:32, :128])
            hps = [pss.tile([32, 256], f32, name=f"heat_ps{j}", tag="pssm")
                   for j in range(2)]
            for i in range(N_HEAT):
                nc.tensor.matmul(hps[i % 2][:], hseed[:, :32], hseed[:],
                                 start=True, stop=True)
            hsink = tp.tile([32, 256], f32, name="hsink", tag="yaf")
            nc.vector.tensor_copy(hsink[:], hps[0][:])
            nc.vector.tensor_copy(hsink[:], hps[1][:])

    nc.finalize()
    return nc


_PROG_CACHE = {}


def _get_program():
    key = (CAP_E0, CAP_S, CAP_E1)
    if key not in _PROG_CACHE:
        _PROG_CACHE[key] = _build_program()
    return _PROG_CACHE[key]


def _pad(a, n, fill, dtype):
    out = np.full((n,), fill, dtype=dtype)
    out[:len(a)] = a.astype(dtype)
    return out


def _bcast(row, p):
    return np.ascontiguousarray(np.broadcast_to(row[None, :].astype(np.float32),
                                                (p, row.shape[0])))


def _core_inputs(src, dst, c):
    Rc = (np.arange(R_PER, dtype=np.int64) + c * R_PER) * NV
    E1 = np.nonzero(np.isin(dst, Rc))[0]
    others = np.setdiff1d(np.unique(src[E1]), Rc)
    S = np.concatenate([Rc, others])
    assert len(E1) <= CAP_E1 and len(S) <= CAP_S, (len(E1), len(S))
    slot = np.full(16000, -1, np.int64)
    slot[S] = np.arange(len(S))
    E0 = np.nonzero(slot[dst] >= 0)[0]
    assert len(E0) <= CAP_E0, len(E0)
    pos = np.full(src.shape[0], -1, np.int64)
    pos[E0] = np.arange(len(E0))
    e0s, e0d = src[E0], dst[E0]
    e1s, e1d = src[E1], dst[E1]
    return {
        "e0_src": _pad(e0s, CAP_E0, 0, np.int32)[:, None],
        "e0_dst": _pad(e0d, CAP_E0, 0, np.int32)[:, None],
        "e0_srcmod": _bcast(_pad(e0s % B, CAP_E0, 0, np.float32), 64),
        "e0_dstmod": _bcast(_pad(e0d % B, CAP_E0, 0, np.float32), 64),
        "e0_sigma": _pad(slot[e0d], CAP_E0, -1, np.float32),
        "s_node": _pad(S, CAP_S, 0, np.int32)[:, None],
        "s_mod": _bcast(_pad(S % B, CAP_S, 0, np.float32), 64),
        "e1_pos": _bcast(_pad(pos[E1], CAP_E1, -1, np.float32), 128),
        "e1_srcslot": _bcast(_pad(slot[e1s], CAP_E1, -1, np.float32), 128),
        "e1_dstslot": _bcast(_pad(slot[e1d], CAP_E1, -1, np.float32), 128),
        "e1_sigma": _pad(slot[e1d], CAP_E1, -1, np.float32),
    }


def _host_inputs(inputs):
    z = np.ascontiguousarray(np.asarray(inputs["z"], np.float32))
    la = np.ascontiguousarray(np.asarray(inputs["look_ats"], np.float32))

    bf = ml_dtypes.bfloat16

    def T(a):
        return np.ascontiguousarray(np.asarray(a, np.float32).T.astype(bf))

    def C(a):
        return np.ascontiguousarray(a.astype(bf)) if a.dtype != bf else a

    w0e0T = np.ascontiguousarray(np.asarray(inputs["p0_ew0"], np.float32).T)
    w0n0T = np.ascontiguousarray(np.asarray(inputs["p0_nw0"], np.float32).T)
    return {
        "z": z, "lookats": la,
        "w0e0_zsrc": C(w0e0T[0:512]),
        "w0e0_zdst": C(w0e0T[515:1027]),
        "w0e0_laA": C(w0e0T[512:515]),
        "w0e0_laB": C(w0e0T[1027:1030]),
        "w0e0_rel": C(w0e0T[1030:1033]),
        "w0e0_wd": C(w0e0T[1033:1034]),
        "w0e1": T(inputs["p0_ew1"]),
        "w0n0_z": C(w0n0T[0:512]),
        "w0n0_la": C(w0n0T[512:515]),
        "w0n0_agg": C(w0n0T[515:1030]),
        "w0n1": T(inputs["p0_nw1"]),
        "w1e0": T(inputs["p1_ew0"]),
        "w1e1": T(inputs["p1_ew1"]),
        "w1n0": T(inputs["p1_nw0"]),
        "w1n1": T(inputs["p1_nw1"]),
        "b_e00": np.asarray(inputs["p0_eb0"], np.float32),
        "b_e01": np.asarray(inputs["p0_eb1"], np.float32),
        "b_n00": np.asarray(inputs["p0_nb0"], np.float32),
        "b_n01": np.asarray(inputs["p0_nb1"], np.float32),
        "b_e10": np.asarray(inputs["p1_eb0"], np.float32),
        "b_e11": np.asarray(inputs["p1_eb1"], np.float32),
        "b_n10": np.asarray(inputs["p1_nb0"], np.float32),
        "b_n11": np.asarray(inputs["p1_nb1"], np.float32),
    }


def make_in_maps(inputs):
    ei = np.asarray(inputs["edge_index"])
    src, dst = ei[0].astype(np.int64), ei[1].astype(np.int64)
    shared = _host_inputs(inputs)
    return [dict(shared, **_core_inputs(src, dst, c)) for c in range(N_CORES)]


def kernel(**inputs):
    nc = _get_program()
    in_maps = make_in_maps(inputs)
    res = run_bass_kernel_spmd(nc, in_maps, core_ids=list(range(N_CORES)))
    out = np.concatenate([res.results[c]["out"] for c in range(N_CORES)], axis=0)
    return out.astype(np.float32)



# revision 12
# speedup vs baseline: 1.4395x; 1.4395x over previous
"""Trainium2 Bass kernel for nn_LookAtMappingNetwork (gnn_message_passing).

Strategy
--------
The module's output only reads the final node features at rows R = {i*250 :
i in 0..63} (``ws = x[::250]``).  Working backwards through the two message
-passing processors, only a small data-dependent subset of edges/nodes can
influence those rows, for ANY edge_index:

    E1 = edges with dst in R          (~6 per graph)   -> proc-1 edge MLP
    S  = R  ∪  src[E1]                (~65 per core)   -> rows where x1 needed
    E0 = edges with dst in S          (~375 per core)  -> proc-0 edge MLP

Segment-mean counts stay exact because E0/E1 contain ALL edges landing on
S/R.  Everything else the reference computes is dead code.  Each of the 8
cores handles 8 output rows (its R_c) fully independently.

Performance layout
------------------
* All FC weights are transposed, pre-scaled by (lr/sqrt(fan_in))*sqrt(2)
  and packed host-side into ONE bf16 tensor of 128-row K-tiles.  Since
  leaky_relu commutes with positive scaling, each layer's activation
  collapses to a single DVE op  out = max(psum, 0.2*psum)  with zero
  scalar-engine work on the critical path.
* Biases enter PSUM as K=1 matmul rows issued FIRST (start=True), so they
  are off the dependence tail; for the e00/n00 layers they ride spare
  partition rows of packed combo K-tiles.
* All host-marshalled metadata (z, %B selectors, slot ids, pre-gathered
  look_at rows) travels in two f32 tensors -> 2 DMA instructions; weights
  in 5 large DMAs split across both HWDGE rings in first-use order.  This
  removes ~45 small DMAs (600ns serial issue each) and 7 serialized
  indirect gathers from the old front-end.
* PSUM->SBUF copies alternate Vector/Scalar engines; transposes stay on
  the PE (bf16 identity matmuls).
* Output is written un-replicated [8, 512]; the x14 ws broadcast happens
  on the host.
"""

import math

import ml_dtypes
import numpy as np

import concourse.bacc as bacc
import concourse.bass as bass
import concourse.mybir as mybir
import concourse.tile as tile
from concourse.bass_utils import run_bass_kernel_spmd
from concourse.masks import make_identity

f32 = mybir.dt.float32
fr = mybir.dt.bfloat16
i32 = mybir.dt.int32
AF = mybir.ActivationFunctionType
OP = mybir.AluOpType

NV = 250
B = 64
D = 512
LR = 0.01
SQ2 = math.sqrt(2.0)
N_CORES = 8
R_PER = B // N_CORES  # output rows per core

CAP_E0 = 384
CAP_S = 128
CAP_E1 = 128
NT0 = CAP_E0 // 128

G_E00 = LR / math.sqrt(1034.0)
G_E01 = LR / math.sqrt(512.0)
G_N00 = LR / math.sqrt(1030.0)
G_N01 = LR / math.sqrt(512.0)
G_E10 = LR / math.sqrt(1536.0)
G_E11 = LR / math.sqrt(512.0)
G_N10 = LR / math.sqrt(1024.0)
G_N11 = LR / math.sqrt(512.0)

# ---- packed weight tile indices (each tile = [128, 512] bf16) ----
T_ZSRC = 0    # 4 tiles: w0e0^T rows 0:512    (z of src)
T_ZDST = 4    # 4 tiles: w0e0^T rows 515:1027 (z of dst)
T_BROWS = 8   # 2 tiles: bias rows (LR*sq2*b) at partitions 0/32/64
T_LARAW = 10  # 1 tile: 0:3 laA-rel | 32:35 laB+rel | 96:97 wd | 97:98 b_e00
T_W0E1 = 11   # 4
T_W0N0Z = 15  # 4: w0n0^T rows 0:512
T_W0N0A = 19  # 4: w0n0^T rows 515:1027 (agg features 0:512)
T_N00C = 23   # 1: 0:3 la | 32:35 agg tail | 64:65 b_n00
T_W0N1 = 24   # 4
T_W1E0 = 28   # 12
T_W1E1 = 40   # 4
T_W1N0 = 44   # 8
T_W1N1 = 52   # 4
NT = 56

# brow j -> (tile offset, partition base): matmul bases must be 0/32/64
BROW_SLOT = {"e01": (0, 0), "n01": (0, 32), "e10": (0, 64),
             "e11": (1, 0), "n10": (1, 32), "n11": (1, 64)}

# ---- meta128 [128, 409] f32 column layout ----
C_SIG = 0      # 3 cols: e0 sigma (dst slot in S) per e-tile
C_E1SIG = 3    # e1 sigma (dst slot in R)
C_E1POS = 4    # 128: e1 -> position in E0
C_E1SRC = 132  # 128: e1 src slot in S
C_E1DST = 260  # 128: e1 dst slot in S
C_LAS = 388    # 3: look_ats[S]
C_LASRC = 391  # 9: look_ats[e0 src], 3 cols per e-tile
C_LADST = 400  # 9: look_ats[e0 dst]
M128F = 416    # padded for DMA row alignment

# ---- meta64 [64, 1408] f32 column layout ----
Z0 = 0         # 512: z
C_SMOD = 512   # 384: e0 src % B
C_DMOD = 896   # 384: e0 dst % B
C_SSEL = 1280  # 128: S % B
M64F = 1408

k4 = [(0, 128), (128, 256), (256, 384), (384, 512)]


def _build_program():
    nc = bacc.Bacc("TRN2", target_bir_lowering=False, debug=False,
                   enable_asserts=False, num_devices=N_CORES)

    wpack_d = nc.dram_tensor("wpack", [NT * 128, 512], fr, kind="ExternalInput")
    m64_d = nc.dram_tensor("m64", [64, M64F], f32, kind="ExternalInput")
    m128_d = nc.dram_tensor("m128", [128, M128F], f32, kind="ExternalInput")
    out_d = nc.dram_tensor("out", [R_PER, D], f32, kind="ExternalOutput")

    with tile.TileContext(nc) as tc, \
            tc.tile_pool(name="w", bufs=1) as wp, \
            tc.tile_pool(name="tmp", bufs=8) as tp, \
            tc.tile_pool(name="psb", bufs=4, space="PSUM") as psb, \
            tc.tile_pool(name="pss", bufs=4, space="PSUM") as pss:

        # ---------------- input DMAs (two HWDGE rings, first-use order) ---
        m128 = wp.tile([128, M128F], f32, name="m128")
        nc.sync.dma_start(m128[:], m128_d[:, :])
        m64 = wp.tile([64, M64F], f32, name="m64")
        nc.scalar.dma_start(m64[:], m64_d[:, :])

        wbig = wp.tile([128, NT, 512], fr, name="wbig")

        def wload(eng, a, b_):
            eng.dma_start(
                wbig[:, a:b_, :],
                wpack_d[128 * a:128 * b_, :].rearrange("(t p) d -> p t d",
                                                       p=128))

        # <=4 tiles per DMA; alternate rings, ordered by first use
        wload(nc.sync, 0, 4)           # zsrc
        wload(nc.scalar, 4, 8)         # zdst
        wload(nc.sync, 8, 11)          # brows, laraw
        wload(nc.scalar, 11, 15)       # w0e1
        wload(nc.sync, 15, 19)         # w0n0 z
        wload(nc.scalar, 19, 23)       # w0n0 agg
        wload(nc.sync, 23, 27)         # n00 combo + w0n1 (3)
        wload(nc.scalar, 27, 28)       # w0n1 (last)
        wload(nc.sync, 28, 32)         # w1e0
        wload(nc.scalar, 32, 36)       # w1e0
        wload(nc.sync, 36, 40)         # w1e0
        wload(nc.scalar, 40, 44)       # w1e1
        wload(nc.sync, 44, 48)         # w1n0
        wload(nc.scalar, 48, 52)       # w1n0
        wload(nc.sync, 52, 56)         # w1n1

        def W(i):
            return wbig[:, i, :]

        # ---------------- constants ----------------
        ident_f = wp.tile([128, 128], f32, name="ident_f")
        make_identity(nc, ident_f[:])
        ident = wp.tile([128, 128], fr, name="ident")
        nc.vector.tensor_copy(ident[:], ident_f[:])
        idents = {fr: ident, f32: ident_f}
        ones_f32 = wp.tile([128, 1], f32, name="ones_f32")
        nc.gpsimd.memset(ones_f32[:], 1.0)
        zeros_f32 = wp.tile([128, 1], f32, name="zeros_f32")
        nc.gpsimd.memset(zeros_f32[:], 0.0)
        iota_free = wp.tile([128, 128], f32, name="iota_free")
        nc.gpsimd.iota(iota_free[:], pattern=[[1, 128]], base=0,
                       channel_multiplier=0, allow_small_or_imprecise_dtypes=True)
        iota_part = []
        for t in range(NT0):
            it = wp.tile([128, 1], f32, name=f"iota_part{t}")
            nc.gpsimd.iota(it[:], pattern=[[1, 1]], base=128 * t,
                           channel_multiplier=1,
                           allow_small_or_imprecise_dtypes=True)
            iota_part.append(it)
        # ones rows at partition bases 0/32/64 (for bias-row matmuls)
        ones_rows = wp.tile([65, 128], fr, name="ones_rows")
        nc.vector.tensor_copy(ones_rows[:], ones_f32[:65, :1].to_broadcast([65, 128]))

        _uid = [0]

        def uid():
            _uid[0] += 1
            return _uid[0]

        def sb(shape, name):
            return wp.tile(shape, fr, name=name)

        _cp = [0]

        def ps_copy(dst_ap, src_ap):
            """PSUM->SBUF copy, alternating Vector/Scalar engines."""
            _cp[0] += 1
            if _cp[0] % 2 == 0:
                nc.vector.tensor_copy(dst_ap, src_ap)
            else:
                nc.scalar.copy(dst_ap, src_ap)

        def copyT(src_ap, p, f, dst_ap):
            """PE transpose src [p, f] -> existing sbuf dst_ap [f, p]."""
            sdt = src_ap.dtype
            ps = pss.tile([f, p], sdt, name=f"psT{uid()}", tag="pssm")
            nc.tensor.transpose(ps[:], src_ap, idents[sdt][:p, :p])
            ps_copy(dst_ap, ps[:])

        def peT(src_ap, p, f, name):
            dst = sb([f, p], name)
            copyT(src_ap, p, f, dst[:])
            return dst

        def brow_mm(ps_t, key, p):
            toff, pbase = BROW_SLOT[key]
            nc.tensor.matmul(ps_t[:], ones_rows[pbase:pbase + 1, :p],
                             wbig[pbase:pbase + 1, T_BROWS + toff, :],
                             start=True, stop=False)

        def lrelu(ps_ap, out_ap):
            """out = leaky_relu(psum, 0.2) -- gain pre-folded into weights.
            (The DVE cannot read two PSUM operands, so stage through SBUF.)"""
            p, n = ps_ap.shape
            t = tp.tile([p, n], f32, name=f"lr{uid()}", tag=f"lr{p}_{n}")
            nc.vector.tensor_copy(t[:], ps_ap)
            nc.vector.scalar_tensor_tensor(out_ap, t[:], 0.2, ps_ap,
                                           op0=OP.mult, op1=OP.max)

        # rhs combo tile for the n00 layer (zero-filled now, rows set later)
        rhs_n00 = sb([128, CAP_S], "rhs_n00")
        nc.vector.tensor_copy(rhs_n00[:],
                              zeros_f32[:, :1].to_broadcast([128, CAP_S]))
        nc.vector.tensor_copy(rhs_n00[64:65, :],
                              ones_f32[:1, :1].to_broadcast([1, CAP_S]))

        # ---------------- z normalization ----------------
        zt = m64[:, Z0:Z0 + 512]
        zsq = tp.tile([64, 512], f32, name="zsq", tag="scr")
        zss = wp.tile([64, 1], f32, name="zss")
        nc.vector.tensor_tensor(zsq[:], zt, zt, op=OP.mult)
        nc.vector.tensor_reduce(zss[:], zsq[:], axis=mybir.AxisListType.X,
                                op=OP.add)
        nc.vector.tensor_scalar(zss[:], zss[:], 1.0 / 512.0, 1e-8,
                                OP.mult, OP.add)
        zsr = wp.tile([64, 1], f32, name="zsr")
        nc.scalar.sqrt(zsr[:], zss[:])
        zrin = wp.tile([64, 1], f32, name="zrin")
        nc.vector.reciprocal(zrin[:], zsr[:])
        znt = sb([64, 512], "znt")
        nc.vector.tensor_scalar_mul(znt[:], zt, zrin[:, :1])

        znT = []
        for k in range(4):
            znT.append(peT(znt[:64, 128 * k:128 * (k + 1)], 64, 128, f"znT{k}"))

        # ---------------- selector matrices (DVE, meta-derived) -----------
        def iseq(out_ap, in_ap, iota_t):
            nc.vector.tensor_scalar(out_ap, in_ap, iota_t, None, OP.is_equal)

        sel0s = sb([64, CAP_E0], "sel0s")
        iseq(sel0s[:], m64[:, C_SMOD:C_SMOD + CAP_E0], iota_part[0][:64, :1])
        sel0d = sb([64, CAP_E0], "sel0d")
        iseq(sel0d[:], m64[:, C_DMOD:C_DMOD + CAP_E0], iota_part[0][:64, :1])
        selS = sb([64, CAP_S], "selS")
        iseq(selS[:], m64[:, C_SSEL:C_SSEL + CAP_S], iota_part[0][:64, :1])
        G0 = []
        for t in range(NT0):
            g = sb([128, 128], f"G0_{t}")
            iseq(g[:], iota_free[:], m128[:, C_SIG + t:C_SIG + t + 1])
            G0.append(g)
        selA = sb([128, CAP_E1], "selA")
        iseq(selA[:], m128[:, C_E1SRC:C_E1SRC + 128], iota_part[0][:, :1])
        selB = sb([128, CAP_E1], "selB")
        iseq(selB[:], m128[:, C_E1DST:C_E1DST + 128], iota_part[0][:, :1])
        selE = []
        for t in range(NT0):
            s_ = sb([128, CAP_E1], f"selE{t}")
            iseq(s_[:], m128[:, C_E1POS:C_E1POS + 128], iota_part[t][:, :1])
            selE.append(s_)
        G1 = sb([CAP_E1, R_PER], "G1")
        iseq(G1[:], iota_free[:, 0:R_PER], m128[:, C_E1SIG:C_E1SIG + 1])

        # ---------------- edge geometry: dist + laRhs ---------------------
        ds3 = tp.tile([128, 3], f32, name="ds3", tag="rel")
        for t in range(NT0):
            lasrc = m128[:, C_LASRC + 3 * t:C_LASRC + 3 * (t + 1)]
            ladst = m128[:, C_LADST + 3 * t:C_LADST + 3 * (t + 1)]
            rel = tp.tile([128, 3], f32, name=f"rel{t}", tag="rel")
            nc.vector.tensor_tensor(rel[:], ladst, lasrc, op=OP.subtract)
            sq = tp.tile([128, 3], f32, name=f"sq{t}", tag="rel")
            nc.vector.tensor_tensor(sq[:], rel[:], rel[:], op=OP.mult)
            nc.vector.tensor_reduce(ds3[:, t:t + 1], sq[:],
                                    axis=mybir.AxisListType.X, op=OP.add)
        dist3 = wp.tile([128, 3], f32, name="dist3")
        nc.scalar.sqrt(dist3[:], ds3[:])

        # laRhs: feature-major rhs [97 used rows, E0] matching laWc layout
        laRhs = sb([128, CAP_E0], "laRhs")
        for t in range(NT0):
            cmb = tp.tile([128, 128], f32, name=f"cmb{t}", tag="cmb")
            nc.vector.tensor_copy(cmb[:],
                                  zeros_f32[:, :1].to_broadcast([128, 128]))
            nc.vector.tensor_copy(cmb[:, 0:3],
                                  m128[:, C_LASRC + 3 * t:C_LASRC + 3 * (t + 1)])
            nc.vector.tensor_copy(cmb[:, 32:35],
                                  m128[:, C_LADST + 3 * t:C_LADST + 3 * (t + 1)])
            nc.vector.tensor_copy(cmb[:, 96:97], dist3[:, t:t + 1])
            nc.vector.tensor_copy(cmb[:, 97:98], ones_f32[:, :1])
            copyT(cmb[:], 128, 128, laRhs[:, 128 * t:128 * (t + 1)])

        # ---------------- zterm + zgS (PE) --------------------------------
        def zterm(base, name):
            ps = psb.tile([64, 512], f32, name=f"ps_{name}", tag="psbig")
            for k in range(4):
                nc.tensor.matmul(ps[:], znT[k][:], W(base + k),
                                 start=(k == 0), stop=(k == 3))
            t_ = sb([64, 512], name)
            ps_copy(t_[:], ps[:])
            return t_

        ztermA = zterm(T_ZSRC, "ztermA")
        ztermB = zterm(T_ZDST, "ztermB")

        zgS = []
        for c in range(4):
            ps = pss.tile([128, CAP_S], f32, name=f"ps_zg{c}", tag="pssm")
            nc.tensor.matmul(ps[:], znt[:64, 128 * c:128 * (c + 1)], selS[:],
                             start=True, stop=True)
            t_ = sb([128, CAP_S], f"zgS{c}")
            ps_copy(t_[:], ps[:])
            zgS.append(t_)

        # ---------------- proc-0 edge MLP layer 1 (feature-major) ---------
        h0 = []
        for c in range(4):
            cs = slice(128 * c, 128 * (c + 1))
            ps = psb.tile([128, CAP_E0], f32, name=f"ps_efp{c}", tag="psbig")
            nc.tensor.matmul(ps[:], wbig[0:98, T_LARAW, cs], laRhs[0:98, :],
                             start=True, stop=False)
            nc.tensor.matmul(ps[:], ztermA[:64, cs], sel0s[:],
                             start=False, stop=False)
            nc.tensor.matmul(ps[:], ztermB[:64, cs], sel0d[:],
                             start=False, stop=True)
            o = sb([128, CAP_E0], f"h0_{c}")
            lrelu(ps[:], o[:])
            h0.append(o)

        # ---------------- proc-0 edge MLP layer 2 (token-major) -----------
        msg = []
        for t in range(NT0):
            m = sb([128, 516], f"msg{t}")
            nc.vector.tensor_copy(m[:, 0:3],
                                  m128[:, C_LADST + 3 * t:C_LADST + 3 * (t + 1)])
            nc.vector.tensor_copy(m[:, 515:516], ones_f32[:, :1])
            es = slice(128 * t, 128 * (t + 1))
            ps = psb.tile([128, 512], f32, name=f"ps_ef0{t}", tag="psbig")
            brow_mm(ps, "e01", 128)
            for k in range(4):
                nc.tensor.matmul(ps[:], h0[k][:, es], W(T_W0E1 + k),
                                 start=False, stop=(k == 3))
            lrelu(ps[:], m[:, 3:515])
            msg.append(m)

        # ---------------- aggregation onto S ------------------------------
        ps_a = psb.tile([128, 512], f32, name="ps_agg0a", tag="psbig")
        ps_b = pss.tile([128, 4], f32, name="ps_agg0b", tag="pssm")
        for t in range(NT0):
            nc.tensor.matmul(ps_a[:], G0[t][:], msg[t][:, 0:512],
                             start=(t == 0), stop=(t == NT0 - 1))
            nc.tensor.matmul(ps_b[:], G0[t][:], msg[t][:, 512:516],
                             start=(t == 0), stop=(t == NT0 - 1))
        cnt = wp.tile([128, 1], f32, name="cnt")
        nc.vector.tensor_scalar(cnt[:], ps_b[:, 3:4], 1.0, None, OP.max)
        rin = wp.tile([128, 1], f32, name="rin")
        nc.vector.reciprocal(rin[:], cnt[:])
        aggtok = sb([128, 515], "aggtok")
        nc.vector.tensor_scalar_mul(aggtok[:, 0:512], ps_a[:, 0:512], rin[:, :1])
        nc.vector.tensor_scalar_mul(aggtok[:, 512:515], ps_b[:, 0:3], rin[:, :1])
        aggT = []
        for c in range(4):
            aggT.append(peT(aggtok[:, 128 * c:128 * (c + 1)], 128, 128,
                            f"aggT{c}"))
        # small rows of the n00 rhs combo: la(S) and the agg tail
        copyT(m128[:, C_LAS:C_LAS + 3], 128, 3, rhs_n00[0:3, :])
        copyT(aggtok[:, 512:515], 128, 3, rhs_n00[32:35, :])

        # ---------------- node MLP 0 -> x1 (token-major, S slots) ---------
        ps = psb.tile([CAP_S, 512], f32, name="ps_n00", tag="psbig")
        for c in range(4):
            nc.tensor.matmul(ps[:], zgS[c][:], W(T_W0N0Z + c),
                             start=(c == 0), stop=False)
        for c in range(4):
            nc.tensor.matmul(ps[:], aggT[c][:], W(T_W0N0A + c),
                             start=False, stop=False)
        nc.tensor.matmul(ps[:], rhs_n00[0:97, :], wbig[0:97, T_N00C, :],
                         start=False, stop=True)
        hn_tok = sb([CAP_S, 512], "hn_tok")
        lrelu(ps[:], hn_tok[:])

        hnT = []
        for c in range(4):
            hnT.append(peT(hn_tok[:, 128 * c:128 * (c + 1)], CAP_S, 128,
                           f"hnT{c}"))

        ps = psb.tile([CAP_S, 512], f32, name="ps_n01", tag="psbig")
        brow_mm(ps, "n01", CAP_S)
        for c in range(4):
            nc.tensor.matmul(ps[:], hnT[c][:], W(T_W0N1 + c),
                             start=False, stop=(c == 3))
        x1tok = sb([CAP_S, 512], "x1tok")
        lrelu(ps[:], x1tok[:])

        # x1 at the R slots, feature-major [128 f, 8]
        x1R = []
        for c in range(4):
            ps_ = pss.tile([128, R_PER], f32, name=f"ps_x1R{c}", tag="pssm")
            nc.tensor.matmul(ps_[:], x1tok[:, 128 * c:128 * (c + 1)],
                             ident[:CAP_S, 0:R_PER], start=True, stop=True)
            t_ = sb([128, R_PER], f"x1R{c}")
            ps_copy(t_[:], ps_[:])
            x1R.append(t_)

        # ---------------- proc-1 edge gathers (selection matmuls) ---------
        def sel_gather(lhsT_fns, sel_tiles, name):
            outs = []
            for c in range(4):
                ps_ = pss.tile([128, CAP_E1], f32, name=f"ps_{name}{c}",
                               tag="pssm")
                for t, s_ in enumerate(sel_tiles):
                    nc.tensor.matmul(ps_[:], lhsT_fns[t](c), s_[:],
                                     start=(t == 0),
                                     stop=(t == len(sel_tiles) - 1))
                o = sb([128, CAP_E1], f"{name}{c}")
                ps_copy(o[:], ps_[:])
                outs.append(o)
            return outs

        x1gA = sel_gather([lambda c: x1tok[:, 128 * c:128 * (c + 1)]], [selA],
                          "x1gA")
        x1gB = sel_gather([lambda c: x1tok[:, 128 * c:128 * (c + 1)]], [selB],
                          "x1gB")
        ef0g = sel_gather(
            [(lambda t: (lambda c: msg[t][:, 3 + 128 * c:3 + 128 * (c + 1)]))(t)
             for t in range(NT0)], selE, "ef0g")

        # ---------------- proc-1 edge MLP (token-major, E1) ---------------
        ps = psb.tile([CAP_E1, 512], f32, name="ps_e10", tag="psbig")
        brow_mm(ps, "e10", CAP_E1)
        for i, grp in enumerate(x1gA + x1gB + ef0g):
            nc.tensor.matmul(ps[:], grp[:], W(T_W1E0 + i),
                             start=False, stop=(i == 11))
        h1tok = sb([CAP_E1, 512], "h1tok")
        lrelu(ps[:], h1tok[:])

        h1T = []
        for c in range(4):
            h1T.append(peT(h1tok[:, 128 * c:128 * (c + 1)], CAP_E1, 128,
                           f"h1T{c}"))
        msg1 = sb([CAP_E1, 514], "msg1")
        nc.vector.tensor_copy(msg1[:, 512:514],
                              ones_f32[:, 0:1].to_broadcast([128, 2]))
        ps = psb.tile([CAP_E1, 512], f32, name="ps_e11", tag="psbig")
        brow_mm(ps, "e11", CAP_E1)
        for c in range(4):
            nc.tensor.matmul(ps[:], h1T[c][:], W(T_W1E1 + c),
                             start=False, stop=(c == 3))
        lrelu(ps[:], msg1[:, 0:512])

        # ---------------- aggregation onto R (8 rows) ---------------------
        ps1 = psb.tile([R_PER, 512], f32, name="ps_agg1", tag="psbig")
        nc.tensor.matmul(ps1[:], G1[:], msg1[:, 0:512], start=True, stop=True)
        ps2 = pss.tile([R_PER, 2], f32, name="ps_agg1b", tag="pssm")
        nc.tensor.matmul(ps2[:], G1[:], msg1[:, 512:514], start=True, stop=True)
        cnt1 = wp.tile([R_PER, 1], f32, name="cnt1")
        nc.vector.tensor_scalar(cnt1[:], ps2[:, 0:1], 1.0, None, OP.max)
        rin1 = wp.tile([R_PER, 1], f32, name="rin1")
        nc.vector.reciprocal(rin1[:], cnt1[:])
        agg1tok = sb([R_PER, 512], "agg1tok")
        nc.vector.tensor_scalar_mul(agg1tok[:], ps1[:], rin1[:, :1])
        agg1T = []
        for c in range(4):
            agg1T.append(peT(agg1tok[:R_PER, 128 * c:128 * (c + 1)], R_PER, 128,
                             f"agg1T{c}"))

        # ---------------- final node MLP (token-major, 8 rows) ------------
        ps = psb.tile([R_PER, 512], f32, name="ps_n10", tag="psbig")
        brow_mm(ps, "n10", R_PER)
        for i, grp in enumerate(x1R + agg1T):
            nc.tensor.matmul(ps[:], grp[:], W(T_W1N0 + i),
                             start=False, stop=(i == 7))
        hftok = sb([R_PER, 512], "hftok")
        lrelu(ps[:], hftok[:])
        hfT = []
        for c in range(4):
            hfT.append(peT(hftok[:R_PER, 128 * c:128 * (c + 1)], R_PER, 128,
                           f"hfT{c}"))
        ps = psb.tile([R_PER, 512], f32, name="ps_n11", tag="psbig")
        brow_mm(ps, "n11", R_PER)
        for c in range(4):
            nc.tensor.matmul(ps[:], hfT[c][:], W(T_W1N1 + c),
                             start=False, stop=(c == 3))
        wstok = wp.tile([R_PER, 512], f32, name="wstok")
        lrelu(ps[:], wstok[:])

        nc.sync.dma_start(out_d[:, :], wstok[:, :])

    nc.finalize()
    return nc


_PROG_CACHE = {}


def _get_program():
    key = (CAP_E0, CAP_S, CAP_E1)
    if key not in _PROG_CACHE:
        _PROG_CACHE[key] = _build_program()
    return _PROG_CACHE[key]


def _pad(a, n, fill):
    out = np.full((n,), fill, dtype=np.float32)
    out[:len(a)] = a.astype(np.float32)
    return out


def _host_weights(inputs):
    """Pack all FC weights (transposed, gain*sqrt2 pre-folded) + biases
    into one [NT*128, 512] bf16 tensor of K-tiles."""
    f = np.float32
    s = SQ2

    def T(name):
        return np.ascontiguousarray(np.asarray(inputs[name], f).T)

    w0e0T, w0e1T = T("p0_ew0"), T("p0_ew1")
    w0n0T, w0n1T = T("p0_nw0"), T("p0_nw1")
    w1e0T, w1e1T = T("p1_ew0"), T("p1_ew1")
    w1n0T, w1n1T = T("p1_nw0"), T("p1_nw1")

    def bias(name):
        return np.asarray(inputs[name], f)

    wpk = np.zeros((NT * 128, 512), f)

    def put(idx, rows):
        wpk[idx * 128: idx * 128 + rows.shape[0]] = rows

    put(T_ZSRC, w0e0T[0:512] * (G_E00 * s))
    put(T_ZDST, w0e0T[515:1027] * (G_E00 * s))
    brows = np.zeros((2 * 128, 512), f)
    for key, bname in [("e01", "p0_eb1"), ("n01", "p0_nb1"),
                       ("e10", "p1_eb0"), ("e11", "p1_eb1"),
                       ("n10", "p1_nb0"), ("n11", "p1_nb1")]:
        toff, pbase = BROW_SLOT[key]
        brows[toff * 128 + pbase] = bias(bname) * (LR * s)
    put(T_BROWS, brows)
    # rel = la[dst]-la[src] folds into the src/dst la blocks:
    #   src rows get (laA - w_rel), dst rows get (laB + w_rel)
    laraw = np.zeros((128, 512), f)
    laraw[0:3] = (w0e0T[512:515] - w0e0T[1030:1033]) * (G_E00 * s)
    laraw[32:35] = (w0e0T[1027:1030] + w0e0T[1030:1033]) * (G_E00 * s)
    laraw[96:97] = w0e0T[1033:1034] * (G_E00 * s)  # dist weight
    laraw[97] = bias("p0_eb0") * (LR * s)
    put(T_LARAW, laraw)
    put(T_W0E1, w0e1T * (G_E01 * s))
    put(T_W0N0Z, w0n0T[0:512] * (G_N00 * s))
    put(T_W0N0A, w0n0T[515:1027] * (G_N00 * s))
    comb = np.zeros((128, 512), f)
    comb[0:3] = w0n0T[512:515] * (G_N00 * s)      # la features of x
    comb[32:35] = w0n0T[1027:1030] * (G_N00 * s)  # agg tail (512:515)
    comb[64] = bias("p0_nb0") * (LR * s)
    put(T_N00C, comb)
    put(T_W0N1, w0n1T * (G_N01 * s))
    put(T_W1E0, w1e0T * (G_E10 * s))
    put(T_W1E1, w1e1T * (G_E11 * s))
    put(T_W1N0, w1n0T * (G_N10 * s))
    put(T_W1N1, w1n1T * (G_N11 * s))
    return np.ascontiguousarray(wpk.astype(ml_dtypes.bfloat16))


def _core_meta(z, la, src, dst, c):
    """Per-core metadata tensors (integer index-set construction + row
    gathers of input data; no arithmetic on tensor values)."""
    Rc = (np.arange(R_PER, dtype=np.int64) + c * R_PER) * NV
    E1 = np.nonzero(np.isin(dst, Rc))[0]
    others = np.setdiff1d(np.unique(src[E1]), Rc)
    S = np.concatenate([Rc, others])
    assert len(E1) <= CAP_E1 and len(S) <= CAP_S, (len(E1), len(S))
    slot = np.full(16000, -1, np.int64)
    slot[S] = np.arange(len(S))
    E0 = np.nonzero(slot[dst] >= 0)[0]
    assert len(E0) <= CAP_E0, len(E0)
    pos = np.full(src.shape[0], -1, np.int64)
    pos[E0] = np.arange(len(E0))
    e0s, e0d = src[E0], dst[E0]
    e1s, e1d = src[E1], dst[E1]

    def gat(idx, n):
        out = np.zeros((n, 3), np.float32)
        out[:len(idx)] = la[idx]
        return out

    m128 = np.zeros((128, M128F), np.float32)
    m128[:, C_SIG:C_SIG + NT0] = _pad(slot[e0d], CAP_E0, -1).reshape(NT0, 128).T
    m128[:, C_E1SIG] = _pad(slot[e1d], CAP_E1, -1)
    m128[:, C_E1POS:C_E1POS + 128] = _pad(pos[E1], CAP_E1, -1)[None, :]
    m128[:, C_E1SRC:C_E1SRC + 128] = _pad(slot[e1s], CAP_E1, -1)[None, :]
    m128[:, C_E1DST:C_E1DST + 128] = _pad(slot[e1d], CAP_E1, -1)[None, :]
    m128[:, C_LAS:C_LAS + 3] = gat(S, CAP_S)
    la_s = gat(e0s, CAP_E0).reshape(NT0, 128, 3)
    la_d = gat(e0d, CAP_E0).reshape(NT0, 128, 3)
    for t in range(NT0):
        m128[:, C_LASRC + 3 * t:C_LASRC + 3 * (t + 1)] = la_s[t]
        m128[:, C_LADST + 3 * t:C_LADST + 3 * (t + 1)] = la_d[t]

    m64 = np.zeros((64, M64F), np.float32)
    m64[:, Z0:Z0 + 512] = z
    m64[:, C_SMOD:C_SMOD + CAP_E0] = _pad(e0s % B, CAP_E0, 0)[None, :]
    m64[:, C_DMOD:C_DMOD + CAP_E0] = _pad(e0d % B, CAP_E0, 0)[None, :]
    m64[:, C_SSEL:C_SSEL + CAP_S] = _pad(S % B, CAP_S, 0)[None, :]
    return {"m64": m64, "m128": np.ascontiguousarray(m128)}


def make_in_maps(inputs):
    ei = np.asarray(inputs["edge_index"])
    src, dst = ei[0].astype(np.int64), ei[1].astype(np.int64)
    z = np.ascontiguousarray(np.asarray(inputs["z"], np.float32))
    la = np.ascontiguousarray(np.asarray(inputs["look_ats"], np.float32))
    wpk = _host_weights(inputs)
    return [dict(wpack=wpk, **_core_meta(z, la, src, dst, c))
            for c in range(N_CORES)]


def kernel(**inputs):
    nc = _get_program()
    in_maps = make_in_maps(inputs)
    res = run_bass_kernel_spmd(nc, in_maps, core_ids=list(range(N_CORES)))
    ws = np.concatenate([res.results[c]["out"] for c in range(N_CORES)],
                        axis=0).astype(np.float32)
    return np.ascontiguousarray(
        np.broadcast_to(ws[:, None, :], (B, 14, D))).astype(np.float32)


# revision 13
# speedup vs baseline: 1.5361x; 1.0671x over previous
"""Trainium2 Bass kernel for nn_LookAtMappingNetwork (gnn_message_passing).

Strategy
--------
The module's output only reads the final node features at rows R = {i*250 :
i in 0..63} (``ws = x[::250]``).  Working backwards through the two message
-passing processors, only a small data-dependent subset of edges/nodes can
influence those rows, for ANY edge_index:

    E1 = edges with dst in R          (~6 per graph)   -> proc-1 edge MLP
    S  = R  ∪  src[E1]                (~65 per core)   -> rows where x1 needed
    E0 = edges with dst in S          (~375 per core)  -> proc-0 edge MLP

Segment-mean counts stay exact because E0/E1 contain ALL edges landing on
S/R.  Everything else the reference computes is dead code.  Each of the 8
cores handles 8 output rows (its R_c) fully independently.

Performance layout
------------------
* All FC weights are transposed, pre-scaled by (lr/sqrt(fan_in))*sqrt(2)
  and packed host-side into ONE bf16 tensor of 128-row K-tiles.  Since
  leaky_relu commutes with positive scaling, each layer's activation
  collapses to a single DVE op  out = max(psum, 0.2*psum)  with zero
  scalar-engine work on the critical path.
* Biases enter PSUM as K=1 matmul rows issued FIRST (start=True), so they
  are off the dependence tail; for the e00/n00 layers they ride spare
  partition rows of packed combo K-tiles.
* All host-marshalled metadata (z, %B selectors, slot ids, pre-gathered
  look_at rows) travels in two f32 tensors -> 2 DMA instructions; weights
  in 5 large DMAs split across both HWDGE rings in first-use order.  This
  removes ~45 small DMAs (600ns serial issue each) and 7 serialized
  indirect gathers from the old front-end.
* PSUM->SBUF copies alternate Vector/Scalar engines; transposes stay on
  the PE (bf16 identity matmuls).
* Output is written un-replicated [8, 512]; the x14 ws broadcast happens
  on the host.
"""

import math

import ml_dtypes
import numpy as np

import concourse.bacc as bacc
import concourse.bass as bass
import concourse.mybir as mybir
import concourse.tile as tile
from concourse.bass_utils import run_bass_kernel_spmd
from concourse.masks import make_identity

f32 = mybir.dt.float32
fr = mybir.dt.bfloat16
i32 = mybir.dt.int32
AF = mybir.ActivationFunctionType
OP = mybir.AluOpType

NV = 250
B = 64
D = 512
LR = 0.01
SQ2 = math.sqrt(2.0)
N_CORES = 8
R_PER = B // N_CORES  # output rows per core

CAP_E0 = 384
CAP_S = 128
CAP_E1 = 128
NT0 = CAP_E0 // 128

G_E00 = LR / math.sqrt(1034.0)
G_E01 = LR / math.sqrt(512.0)
G_N00 = LR / math.sqrt(1030.0)
G_N01 = LR / math.sqrt(512.0)
G_E10 = LR / math.sqrt(1536.0)
G_E11 = LR / math.sqrt(512.0)
G_N10 = LR / math.sqrt(1024.0)
G_N11 = LR / math.sqrt(512.0)

# ---- packed weight tile indices (each tile = [128, 512] bf16) ----
# PAD tiles sit at group boundaries: each weight-DMA group rewrites the
# previous group's pad tile, giving a WAW hazard that makes the Tile
# scheduler chain the transfers in first-use order (otherwise the SDMA
# engines round-robin ALL queued DMAs and the first weights arrive last).
T_ZSRC = 0    # 4 tiles: w0e0^T rows 0:512    (z of src)
T_ZDST = 4    # 4 tiles: w0e0^T rows 515:1027 (z of dst)
T_BROWS = 8   # 2 tiles: bias rows (LR*sq2*b) at partitions 0/32/64
T_LARAW = 10  # 1 tile: 0:3 laA-rel | 32:35 laB+rel | 96:97 wd | 97:98 b_e00
# pad 11
T_W0E1 = 12   # 4
T_W0N0Z = 16  # 4: w0n0^T rows 0:512
T_W0N0A = 20  # 4: w0n0^T rows 515:1027 (agg features 0:512)
T_N00C = 24   # 1: 0:3 la | 32:35 agg tail | 64:65 b_n00
T_W0N1 = 25   # 4
# pad 29
T_W1E0 = 30   # 12
T_W1E1 = 42   # 4
# pad 46
T_W1N0 = 47   # 8
T_W1N1 = 55   # 4
NT = 59

# brow j -> (tile offset, partition base): matmul bases must be 0/32/64
BROW_SLOT = {"e01": (0, 0), "n01": (0, 32), "e10": (0, 64),
             "e11": (1, 0), "n10": (1, 32), "n11": (1, 64)}

# ---- meta128 [128, 409] f32 column layout ----
C_SIG = 0      # 3 cols: e0 sigma (dst slot in S) per e-tile
C_E1SIG = 3    # e1 sigma (dst slot in R)
C_E1POS = 4    # 128: e1 -> position in E0
C_E1SRC = 132  # 128: e1 src slot in S
C_E1DST = 260  # 128: e1 dst slot in S
C_LAS = 388    # 3: look_ats[S]
C_LASRC = 391  # 9: look_ats[e0 src], 3 cols per e-tile
C_LADST = 400  # 9: look_ats[e0 dst]
M128F = 416    # padded for DMA row alignment

# ---- meta64 [64, 1408] f32 column layout ----
Z0 = 0         # 512: z
C_SMOD = 512   # 384: e0 src % B
C_DMOD = 896   # 384: e0 dst % B
C_SSEL = 1280  # 128: S % B
M64F = 1408

k4 = [(0, 128), (128, 256), (256, 384), (384, 512)]


def _build_program():
    nc = bacc.Bacc("TRN2", target_bir_lowering=False, debug=False,
                   enable_asserts=False, num_devices=N_CORES)

    wpack_d = nc.dram_tensor("wpack", [NT * 128, 512], fr, kind="ExternalInput")
    m64_d = nc.dram_tensor("m64", [64, M64F], f32, kind="ExternalInput")
    m128_d = nc.dram_tensor("m128", [128, M128F], f32, kind="ExternalInput")
    out_d = nc.dram_tensor("out", [R_PER, D], f32, kind="ExternalOutput")

    with tile.TileContext(nc) as tc, \
            tc.tile_pool(name="w", bufs=1) as wp, \
            tc.tile_pool(name="tmp", bufs=8) as tp, \
            tc.tile_pool(name="psb", bufs=4, space="PSUM") as psb, \
            tc.tile_pool(name="pss", bufs=4, space="PSUM") as pss:

        # ---------------- input DMAs (two HWDGE rings, first-use order) ---
        m128 = wp.tile([128, M128F], f32, name="m128")
        nc.sync.dma_start(m128[:], m128_d[:, :])
        m64 = wp.tile([64, M64F], f32, name="m64")
        nc.scalar.dma_start(m64[:], m64_d[:, :])

        wbig = wp.tile([128, NT, 512], fr, name="wbig")

        def wload(eng, a, b_):
            eng.dma_start(
                wbig[:, a:b_, :],
                wpack_d[128 * a:128 * b_, :].rearrange("(t p) d -> p t d",
                                                       p=128))

        # One chained stream on the sync ring, in first-use order; each
        # group overlaps the previous group's pad tile (WAW -> serialized).
        wload(nc.sync, 0, 12)          # zsrc, zdst, brows, laraw, pad11
        wload(nc.sync, 11, 30)         # w0e1, w0n0, n00 combo, w0n1, pad29
        wload(nc.sync, 29, 47)         # w1e0, w1e1, pad46
        wload(nc.sync, 46, 59)         # w1n0, w1n1

        def W(i):
            return wbig[:, i, :]

        # ---------------- constants ----------------
        ident_f = wp.tile([128, 128], f32, name="ident_f")
        make_identity(nc, ident_f[:])
        ident = wp.tile([128, 128], fr, name="ident")
        nc.vector.tensor_copy(ident[:], ident_f[:])
        idents = {fr: ident, f32: ident_f}
        ones_f32 = wp.tile([128, 1], f32, name="ones_f32")
        nc.gpsimd.memset(ones_f32[:], 1.0)
        zeros_f32 = wp.tile([128, 1], f32, name="zeros_f32")
        nc.gpsimd.memset(zeros_f32[:], 0.0)
        iota_free = wp.tile([128, 128], f32, name="iota_free")
        nc.gpsimd.iota(iota_free[:], pattern=[[1, 128]], base=0,
                       channel_multiplier=0, allow_small_or_imprecise_dtypes=True)
        iota_part = []
        for t in range(NT0):
            it = wp.tile([128, 1], f32, name=f"iota_part{t}")
            nc.gpsimd.iota(it[:], pattern=[[1, 1]], base=128 * t,
                           channel_multiplier=1,
                           allow_small_or_imprecise_dtypes=True)
            iota_part.append(it)
        # ones rows at partition bases 0/32/64 (for bias-row matmuls)
        ones_rows = wp.tile([65, 128], fr, name="ones_rows")
        nc.vector.tensor_copy(ones_rows[:], ones_f32[:65, :1].to_broadcast([65, 128]))

        _uid = [0]

        def uid():
            _uid[0] += 1
            return _uid[0]

        def sb(shape, name):
            return wp.tile(shape, fr, name=name)

        _cp = [0]

        def ps_copy(dst_ap, src_ap):
            """PSUM->SBUF copy, alternating Vector/Scalar engines."""
            _cp[0] += 1
            if _cp[0] % 2 == 0:
                nc.vector.tensor_copy(dst_ap, src_ap)
            else:
                nc.scalar.copy(dst_ap, src_ap)

        def copyT(src_ap, p, f, dst_ap):
            """PE transpose src [p, f] -> existing sbuf dst_ap [f, p]."""
            sdt = src_ap.dtype
            ps = pss.tile([f, p], sdt, name=f"psT{uid()}", tag="pssm")
            nc.tensor.transpose(ps[:], src_ap, idents[sdt][:p, :p])
            ps_copy(dst_ap, ps[:])

        def peT(src_ap, p, f, name):
            dst = sb([f, p], name)
            copyT(src_ap, p, f, dst[:])
            return dst

        def brow_mm(ps_t, key, p):
            toff, pbase = BROW_SLOT[key]
            nc.tensor.matmul(ps_t[:], ones_rows[pbase:pbase + 1, :p],
                             wbig[pbase:pbase + 1, T_BROWS + toff, :],
                             start=True, stop=False)

        def lrelu(ps_ap, out_ap):
            """out = leaky_relu(psum, 0.2) -- gain pre-folded into weights.
            (The DVE cannot read two PSUM operands, so stage through SBUF.)"""
            p, n = ps_ap.shape
            t = tp.tile([p, n], f32, name=f"lr{uid()}", tag=f"lr{p}_{n}")
            nc.vector.tensor_copy(t[:], ps_ap)
            nc.vector.scalar_tensor_tensor(out_ap, t[:], 0.2, ps_ap,
                                           op0=OP.mult, op1=OP.max)

        def lrelu_chunk(ps_t, out_t, p, consume):
            """Chunked lrelu over 4 x 128 output columns; consume(c, out_ap)
            emits the chunk's consumers right away so the PE restarts while
            later chunks are still on the DVE.  First chunk's copy runs on
            Vector (lowest latency), the rest on Scalar in parallel."""
            for c in range(4):
                cs = slice(128 * c, 128 * (c + 1))
                t = tp.tile([p, 128], f32, name=f"lrc{uid()}", tag=f"lrc{p}")
                if c == 0:
                    nc.vector.tensor_copy(t[:], ps_t[:, cs])
                else:
                    nc.scalar.copy(t[:], ps_t[:, cs])
                nc.vector.scalar_tensor_tensor(out_t[:, cs], t[:], 0.2,
                                               ps_t[:, cs],
                                               op0=OP.mult, op1=OP.max)
                consume(c, out_t[:, cs])

        # rhs combo tile for the n00 layer (zero-filled now, rows set later)
        rhs_n00 = sb([128, CAP_S], "rhs_n00")
        nc.vector.tensor_copy(rhs_n00[:],
                              zeros_f32[:, :1].to_broadcast([128, CAP_S]))
        nc.vector.tensor_copy(rhs_n00[64:65, :],
                              ones_f32[:1, :1].to_broadcast([1, CAP_S]))

        # ---------------- z normalization ----------------
        zt = m64[:, Z0:Z0 + 512]
        zsq = tp.tile([64, 512], f32, name="zsq", tag="scr")
        zss = wp.tile([64, 1], f32, name="zss")
        nc.vector.tensor_tensor(zsq[:], zt, zt, op=OP.mult)
        nc.vector.tensor_reduce(zss[:], zsq[:], axis=mybir.AxisListType.X,
                                op=OP.add)
        nc.vector.tensor_scalar(zss[:], zss[:], 1.0 / 512.0, 1e-8,
                                OP.mult, OP.add)
        zsr = wp.tile([64, 1], f32, name="zsr")
        nc.scalar.sqrt(zsr[:], zss[:])
        zrin = wp.tile([64, 1], f32, name="zrin")
        nc.vector.reciprocal(zrin[:], zsr[:])
        znt = sb([64, 512], "znt")
        nc.vector.tensor_scalar_mul(znt[:], zt, zrin[:, :1])

        znT = []
        for k in range(4):
            znT.append(peT(znt[:64, 128 * k:128 * (k + 1)], 64, 128, f"znT{k}"))

        # ---------------- selector matrices (DVE, meta-derived) -----------
        def iseq(out_ap, in_ap, iota_t):
            nc.vector.tensor_scalar(out_ap, in_ap, iota_t, None, OP.is_equal)

        sel0s = sb([64, CAP_E0], "sel0s")
        iseq(sel0s[:], m64[:, C_SMOD:C_SMOD + CAP_E0], iota_part[0][:64, :1])
        sel0d = sb([64, CAP_E0], "sel0d")
        iseq(sel0d[:], m64[:, C_DMOD:C_DMOD + CAP_E0], iota_part[0][:64, :1])
        selS = sb([64, CAP_S], "selS")
        iseq(selS[:], m64[:, C_SSEL:C_SSEL + CAP_S], iota_part[0][:64, :1])
        G0 = []
        for t in range(NT0):
            g = sb([128, 128], f"G0_{t}")
            iseq(g[:], iota_free[:], m128[:, C_SIG + t:C_SIG + t + 1])
            G0.append(g)
        selA = sb([128, CAP_E1], "selA")
        iseq(selA[:], m128[:, C_E1SRC:C_E1SRC + 128], iota_part[0][:, :1])
        selB = sb([128, CAP_E1], "selB")
        iseq(selB[:], m128[:, C_E1DST:C_E1DST + 128], iota_part[0][:, :1])
        selE = []
        for t in range(NT0):
            s_ = sb([128, CAP_E1], f"selE{t}")
            iseq(s_[:], m128[:, C_E1POS:C_E1POS + 128], iota_part[t][:, :1])
            selE.append(s_)
        G1 = sb([CAP_E1, R_PER], "G1")
        iseq(G1[:], iota_free[:, 0:R_PER], m128[:, C_E1SIG:C_E1SIG + 1])

        # ---------------- edge geometry: dist + laRhs ---------------------
        ds3 = tp.tile([128, 3], f32, name="ds3", tag="rel")
        for t in range(NT0):
            lasrc = m128[:, C_LASRC + 3 * t:C_LASRC + 3 * (t + 1)]
            ladst = m128[:, C_LADST + 3 * t:C_LADST + 3 * (t + 1)]
            rel = tp.tile([128, 3], f32, name=f"rel{t}", tag="rel")
            nc.vector.tensor_tensor(rel[:], ladst, lasrc, op=OP.subtract)
            sq = tp.tile([128, 3], f32, name=f"sq{t}", tag="rel")
            nc.vector.tensor_tensor(sq[:], rel[:], rel[:], op=OP.mult)
            nc.vector.tensor_reduce(ds3[:, t:t + 1], sq[:],
                                    axis=mybir.AxisListType.X, op=OP.add)
        dist3 = wp.tile([128, 3], f32, name="dist3")
        nc.scalar.sqrt(dist3[:], ds3[:])

        # laRhs: feature-major rhs [97 used rows, E0] matching laWc layout
        laRhs = sb([128, CAP_E0], "laRhs")
        for t in range(NT0):
            cmb = tp.tile([128, 128], f32, name=f"cmb{t}", tag="cmb")
            nc.vector.tensor_copy(cmb[:],
                                  zeros_f32[:, :1].to_broadcast([128, 128]))
            nc.vector.tensor_copy(cmb[:, 0:3],
                                  m128[:, C_LASRC + 3 * t:C_LASRC + 3 * (t + 1)])
            nc.vector.tensor_copy(cmb[:, 32:35],
                                  m128[:, C_LADST + 3 * t:C_LADST + 3 * (t + 1)])
            nc.vector.tensor_copy(cmb[:, 96:97], dist3[:, t:t + 1])
            nc.vector.tensor_copy(cmb[:, 97:98], ones_f32[:, :1])
            copyT(cmb[:], 128, 128, laRhs[:, 128 * t:128 * (t + 1)])

        # ---------------- zterm + zgS (PE) --------------------------------
        def zterm(base, name):
            ps = psb.tile([64, 512], f32, name=f"ps_{name}", tag="psbig")
            for k in range(4):
                nc.tensor.matmul(ps[:], znT[k][:], W(base + k),
                                 start=(k == 0), stop=(k == 3))
            t_ = sb([64, 512], name)
            ps_copy(t_[:], ps[:])
            return t_

        ztermA = zterm(T_ZSRC, "ztermA")
        ztermB = zterm(T_ZDST, "ztermB")

        zgS = []
        for c in range(4):
            ps = pss.tile([128, CAP_S], f32, name=f"ps_zg{c}", tag="pssm")
            nc.tensor.matmul(ps[:], znt[:64, 128 * c:128 * (c + 1)], selS[:],
                             start=True, stop=True)
            t_ = sb([128, CAP_S], f"zgS{c}")
            ps_copy(t_[:], ps[:])
            zgS.append(t_)

        # ---------------- proc-0 edge MLP layer 1 (feature-major) ---------
        h0 = []
        for c in range(4):
            cs = slice(128 * c, 128 * (c + 1))
            ps = psb.tile([128, CAP_E0], f32, name=f"ps_efp{c}", tag="psbig")
            nc.tensor.matmul(ps[:], wbig[0:98, T_LARAW, cs], laRhs[0:98, :],
                             start=True, stop=False)
            nc.tensor.matmul(ps[:], ztermA[:64, cs], sel0s[:],
                             start=False, stop=False)
            nc.tensor.matmul(ps[:], ztermB[:64, cs], sel0d[:],
                             start=False, stop=True)
            o = sb([128, CAP_E0], f"h0_{c}")
            lrelu(ps[:], o[:])
            h0.append(o)

        # ---------------- proc-0 edge MLP layer 2 (token-major) -----------
        msg = []
        for t in range(NT0):
            m = sb([128, 516], f"msg{t}")
            nc.vector.tensor_copy(m[:, 0:3],
                                  m128[:, C_LADST + 3 * t:C_LADST + 3 * (t + 1)])
            nc.vector.tensor_copy(m[:, 515:516], ones_f32[:, :1])
            es = slice(128 * t, 128 * (t + 1))
            ps = psb.tile([128, 512], f32, name=f"ps_ef0{t}", tag="psbig")
            brow_mm(ps, "e01", 128)
            for k in range(4):
                nc.tensor.matmul(ps[:], h0[k][:, es], W(T_W0E1 + k),
                                 start=False, stop=(k == 3))
            lrelu(ps[:], m[:, 3:515])
            msg.append(m)

        # ---------------- aggregation onto S ------------------------------
        ps_a = psb.tile([128, 512], f32, name="ps_agg0a", tag="psbig")
        ps_b = pss.tile([128, 4], f32, name="ps_agg0b", tag="pssm")
        for t in range(NT0):
            nc.tensor.matmul(ps_a[:], G0[t][:], msg[t][:, 0:512],
                             start=(t == 0), stop=(t == NT0 - 1))
            nc.tensor.matmul(ps_b[:], G0[t][:], msg[t][:, 512:516],
                             start=(t == 0), stop=(t == NT0 - 1))
        cnt = wp.tile([128, 1], f32, name="cnt")
        nc.vector.tensor_scalar(cnt[:], ps_b[:, 3:4], 1.0, None, OP.max)
        rin = wp.tile([128, 1], f32, name="rin")
        nc.vector.reciprocal(rin[:], cnt[:])
        aggtok = sb([128, 515], "aggtok")
        nc.vector.tensor_scalar_mul(aggtok[:, 0:512], ps_a[:, 0:512], rin[:, :1])
        nc.vector.tensor_scalar_mul(aggtok[:, 512:515], ps_b[:, 0:3], rin[:, :1])
        aggT = []
        for c in range(4):
            aggT.append(peT(aggtok[:, 128 * c:128 * (c + 1)], 128, 128,
                            f"aggT{c}"))
        # small rows of the n00 rhs combo: la(S) and the agg tail
        copyT(m128[:, C_LAS:C_LAS + 3], 128, 3, rhs_n00[0:3, :])
        copyT(aggtok[:, 512:515], 128, 3, rhs_n00[32:35, :])

        # ---------------- node MLP 0 -> x1 (token-major, S slots) ---------
        ps = psb.tile([CAP_S, 512], f32, name="ps_n00", tag="psbig")
        for c in range(4):
            nc.tensor.matmul(ps[:], zgS[c][:], W(T_W0N0Z + c),
                             start=(c == 0), stop=False)
        for c in range(4):
            nc.tensor.matmul(ps[:], aggT[c][:], W(T_W0N0A + c),
                             start=False, stop=False)
        nc.tensor.matmul(ps[:], rhs_n00[0:97, :], wbig[0:97, T_N00C, :],
                         start=False, stop=True)
        hn_tok = sb([CAP_S, 512], "hn_tok")
        hnT = [sb([128, CAP_S], f"hnT{c}") for c in range(4)]
        lrelu_chunk(ps[:], hn_tok[:], CAP_S,
                    lambda c, ap: copyT(ap, CAP_S, 128, hnT[c][:]))

        ps = psb.tile([CAP_S, 512], f32, name="ps_n01", tag="psbig")
        brow_mm(ps, "n01", CAP_S)
        for c in range(4):
            nc.tensor.matmul(ps[:], hnT[c][:], W(T_W0N1 + c),
                             start=False, stop=(c == 3))
        x1tok = sb([CAP_S, 512], "x1tok")
        x1R = [sb([128, R_PER], f"x1R{c}") for c in range(4)]

        def x1_consume(c, ap):
            ps_ = pss.tile([128, R_PER], f32, name=f"ps_x1R{c}", tag="pssm")
            nc.tensor.matmul(ps_[:], ap, ident[:CAP_S, 0:R_PER],
                             start=True, stop=True)
            ps_copy(x1R[c][:], ps_[:])

        lrelu_chunk(ps[:], x1tok[:], CAP_S, x1_consume)

        # ---------------- proc-1 edge gathers (selection matmuls) ---------
        def sel_gather(lhsT_fns, sel_tiles, name):
            outs = []
            for c in range(4):
                ps_ = pss.tile([128, CAP_E1], f32, name=f"ps_{name}{c}",
                               tag="pssm")
                for t, s_ in enumerate(sel_tiles):
                    nc.tensor.matmul(ps_[:], lhsT_fns[t](c), s_[:],
                                     start=(t == 0),
                                     stop=(t == len(sel_tiles) - 1))
                o = sb([128, CAP_E1], f"{name}{c}")
                ps_copy(o[:], ps_[:])
                outs.append(o)
            return outs

        x1gA = sel_gather([lambda c: x1tok[:, 128 * c:128 * (c + 1)]], [selA],
                          "x1gA")
        x1gB = sel_gather([lambda c: x1tok[:, 128 * c:128 * (c + 1)]], [selB],
                          "x1gB")
        ef0g = sel_gather(
            [(lambda t: (lambda c: msg[t][:, 3 + 128 * c:3 + 128 * (c + 1)]))(t)
             for t in range(NT0)], selE, "ef0g")

        # ---------------- proc-1 edge MLP (token-major, E1) ---------------
        ps = psb.tile([CAP_E1, 512], f32, name="ps_e10", tag="psbig")
        brow_mm(ps, "e10", CAP_E1)
        for i, grp in enumerate(x1gA + x1gB + ef0g):
            nc.tensor.matmul(ps[:], grp[:], W(T_W1E0 + i),
                             start=False, stop=(i == 11))
        h1tok = sb([CAP_E1, 512], "h1tok")
        h1T = [sb([128, CAP_E1], f"h1T{c}") for c in range(4)]
        lrelu_chunk(ps[:], h1tok[:], CAP_E1,
                    lambda c, ap: copyT(ap, CAP_E1, 128, h1T[c][:]))
        msg1 = sb([CAP_E1, 514], "msg1")
        nc.vector.tensor_copy(msg1[:, 512:514],
                              ones_f32[:, 0:1].to_broadcast([128, 2]))
        ps = psb.tile([CAP_E1, 512], f32, name="ps_e11", tag="psbig")
        brow_mm(ps, "e11", CAP_E1)
        for c in range(4):
            nc.tensor.matmul(ps[:], h1T[c][:], W(T_W1E1 + c),
                             start=False, stop=(c == 3))
        lrelu(ps[:], msg1[:, 0:512])

        # ---------------- aggregation onto R (8 rows) ---------------------
        ps1 = psb.tile([R_PER, 512], f32, name="ps_agg1", tag="psbig")
        nc.tensor.matmul(ps1[:], G1[:], msg1[:, 0:512], start=True, stop=True)
        ps2 = pss.tile([R_PER, 2], f32, name="ps_agg1b", tag="pssm")
        nc.tensor.matmul(ps2[:], G1[:], msg1[:, 512:514], start=True, stop=True)
        cnt1 = wp.tile([R_PER, 1], f32, name="cnt1")
        nc.vector.tensor_scalar(cnt1[:], ps2[:, 0:1], 1.0, None, OP.max)
        rin1 = wp.tile([R_PER, 1], f32, name="rin1")
        nc.vector.reciprocal(rin1[:], cnt1[:])
        agg1tok = sb([R_PER, 512], "agg1tok")
        nc.vector.tensor_scalar_mul(agg1tok[:], ps1[:], rin1[:, :1])
        agg1T = []
        for c in range(4):
            agg1T.append(peT(agg1tok[:R_PER, 128 * c:128 * (c + 1)], R_PER, 128,
                             f"agg1T{c}"))

        # ---------------- final node MLP (token-major, 8 rows) ------------
        ps = psb.tile([R_PER, 512], f32, name="ps_n10", tag="psbig")
        brow_mm(ps, "n10", R_PER)
        for i, grp in enumerate(x1R + agg1T):
            nc.tensor.matmul(ps[:], grp[:], W(T_W1N0 + i),
                             start=False, stop=(i == 7))
        hftok = sb([R_PER, 512], "hftok")
        hfT = [sb([128, R_PER], f"hfT{c}") for c in range(4)]
        lrelu_chunk(ps[:], hftok[:], R_PER,
                    lambda c, ap: copyT(ap, R_PER, 128, hfT[c][:]))
        ps = psb.tile([R_PER, 512], f32, name="ps_n11", tag="psbig")
        brow_mm(ps, "n11", R_PER)
        for c in range(4):
            nc.tensor.matmul(ps[:], hfT[c][:], W(T_W1N1 + c),
                             start=False, stop=(c == 3))
        wstok = wp.tile([R_PER, 512], f32, name="wstok")
        lrelu(ps[:], wstok[:])

        nc.sync.dma_start(out_d[:, :], wstok[:, :])

    nc.finalize()
    return nc


_PROG_CACHE = {}


def _get_program():
    key = (CAP_E0, CAP_S, CAP_E1)
    if key not in _PROG_CACHE:
        _PROG_CACHE[key] = _build_program()
    return _PROG_CACHE[key]


def _pad(a, n, fill):
    out = np.full((n,), fill, dtype=np.float32)
    out[:len(a)] = a.astype(np.float32)
    return out


def _host_weights(inputs):
    """Pack all FC weights (transposed, gain*sqrt2 pre-folded) + biases
    into one [NT*128, 512] bf16 tensor of K-tiles."""
    f = np.float32
    s = SQ2

    def T(name):
        return np.ascontiguousarray(np.asarray(inputs[name], f).T)

    w0e0T, w0e1T = T("p0_ew0"), T("p0_ew1")
    w0n0T, w0n1T = T("p0_nw0"), T("p0_nw1")
    w1e0T, w1e1T = T("p1_ew0"), T("p1_ew1")
    w1n0T, w1n1T = T("p1_nw0"), T("p1_nw1")

    def bias(name):
        return np.asarray(inputs[name], f)

    wpk = np.zeros((NT * 128, 512), f)

    def put(idx, rows):
        wpk[idx * 128: idx * 128 + rows.shape[0]] = rows

    put(T_ZSRC, w0e0T[0:512] * (G_E00 * s))
    put(T_ZDST, w0e0T[515:1027] * (G_E00 * s))
    brows = np.zeros((2 * 128, 512), f)
    for key, bname in [("e01", "p0_eb1"), ("n01", "p0_nb1"),
                       ("e10", "p1_eb0"), ("e11", "p1_eb1"),
                       ("n10", "p1_nb0"), ("n11", "p1_nb1")]:
        toff, pbase = BROW_SLOT[key]
        brows[toff * 128 + pbase] = bias(bname) * (LR * s)
    put(T_BROWS, brows)
    # rel = la[dst]-la[src] folds into the src/dst la blocks:
    #   src rows get (laA - w_rel), dst rows get (laB + w_rel)
    laraw = np.zeros((128, 512), f)
    laraw[0:3] = (w0e0T[512:515] - w0e0T[1030:1033]) * (G_E00 * s)
    laraw[32:35] = (w0e0T[1027:1030] + w0e0T[1030:1033]) * (G_E00 * s)
    laraw[96:97] = w0e0T[1033:1034] * (G_E00 * s)  # dist weight
    laraw[97] = bias("p0_eb0") * (LR * s)
    put(T_LARAW, laraw)
    put(T_W0E1, w0e1T * (G_E01 * s))
    put(T_W0N0Z, w0n0T[0:512] * (G_N00 * s))
    put(T_W0N0A, w0n0T[515:1027] * (G_N00 * s))
    comb = np.zeros((128, 512), f)
    comb[0:3] = w0n0T[512:515] * (G_N00 * s)      # la features of x
    comb[32:35] = w0n0T[1027:1030] * (G_N00 * s)  # agg tail (512:515)
    comb[64] = bias("p0_nb0") * (LR * s)
    put(T_N00C, comb)
    put(T_W0N1, w0n1T * (G_N01 * s))
    put(T_W1E0, w1e0T * (G_E10 * s))
    put(T_W1E1, w1e1T * (G_E11 * s))
    put(T_W1N0, w1n0T * (G_N10 * s))
    put(T_W1N1, w1n1T * (G_N11 * s))
    return np.ascontiguousarray(wpk.astype(ml_dtypes.bfloat16))


def _core_meta(z, la, src, dst, c):
    """Per-core metadata tensors (integer index-set construction + row
    gathers of input data; no arithmetic on tensor values)."""
    Rc = (np.arange(R_PER, dtype=np.int64) + c * R_PER) * NV
    E1 = np.nonzero(np.isin(dst, Rc))[0]
    others = np.setdiff1d(np.unique(src[E1]), Rc)
    S = np.concatenate([Rc, others])
    assert len(E1) <= CAP_E1 and len(S) <= CAP_S, (len(E1), len(S))
    slot = np.full(16000, -1, np.int64)
    slot[S] = np.arange(len(S))
    E0 = np.nonzero(slot[dst] >= 0)[0]
    assert len(E0) <= CAP_E0, len(E0)
    pos = np.full(src.shape[0], -1, np.int64)
    pos[E0] = np.arange(len(E0))
    e0s, e0d = src[E0], dst[E0]
    e1s, e1d = src[E1], dst[E1]

    def gat(idx, n):
        out = np.zeros((n, 3), np.float32)
        out[:len(idx)] = la[idx]
        return out

    m128 = np.zeros((128, M128F), np.float32)
    m128[:, C_SIG:C_SIG + NT0] = _pad(slot[e0d], CAP_E0, -1).reshape(NT0, 128).T
    m128[:, C_E1SIG] = _pad(slot[e1d], CAP_E1, -1)
    m128[:, C_E1POS:C_E1POS + 128] = _pad(pos[E1], CAP_E1, -1)[None, :]
    m128[:, C_E1SRC:C_E1SRC + 128] = _pad(slot[e1s], CAP_E1, -1)[None, :]
    m128[:, C_E1DST:C_E1DST + 128] = _pad(slot[e1d], CAP_E1, -1)[None, :]
    m128[:, C_LAS:C_LAS + 3] = gat(S, CAP_S)
    la_s = gat(e0s, CAP_E0).reshape(NT0, 128, 3)
    la_d = gat(e0d, CAP_E0).reshape(NT0, 128, 3)
    for t in range(NT0):
        m128[:, C_LASRC + 3 * t:C_LASRC + 3 * (t + 1)] = la_s[t]
        m128[:, C_LADST + 3 * t:C_LADST + 3 * (t + 1)] = la_d[t]

    m64 = np.zeros((64, M64F), np.float32)
    m64[:, Z0:Z0 + 512] = z
    m64[:, C_SMOD:C_SMOD + CAP_E0] = _pad(e0s % B, CAP_E0, 0)[None, :]
    m64[:, C_DMOD:C_DMOD + CAP_E0] = _pad(e0d % B, CAP_E0, 0)[None, :]
    m64[:, C_SSEL:C_SSEL + CAP_S] = _pad(S % B, CAP_S, 0)[None, :]
    return {"m64": m64, "m128": np.ascontiguousarray(m128)}


def make_in_maps(inputs):
    ei = np.asarray(inputs["edge_index"])
    src, dst = ei[0].astype(np.int64), ei[1].astype(np.int64)
    z = np.ascontiguousarray(np.asarray(inputs["z"], np.float32))
    la = np.ascontiguousarray(np.asarray(inputs["look_ats"], np.float32))
    wpk = _host_weights(inputs)
    return [dict(wpack=wpk, **_core_meta(z, la, src, dst, c))
            for c in range(N_CORES)]


def kernel(**inputs):
    nc = _get_program()
    in_maps = make_in_maps(inputs)
    res = run_bass_kernel_spmd(nc, in_maps, core_ids=list(range(N_CORES)))
    ws = np.concatenate([res.results[c]["out"] for c in range(N_CORES)],
                        axis=0).astype(np.float32)
    return np.ascontiguousarray(
        np.broadcast_to(ws[:, None, :], (B, 14, D))).astype(np.float32)
